# revision 1
# baseline (speedup 1.0000x reference)
"""DevignModel (GGNN + conv head) Trainium2 Bass kernel, 8-core SPMD.

Sharding: nodes/graphs split 8 ways (8192 nodes = 16 graphs per core).
Per GGNN step: each core computes its message shard m = h @ W (fp32r),
casts to bf16, AllGathers the full message table, then aggregates its
local edges with a 4-queue dma_gather (pair-row table, int16-safe) and
a PE weighted-indicator matmul that yields aggT (transposed) directly.
GRU runs in fp32r with ACT sigmoid/tanh. The conv/BN/MLP head runs per
graph on-core with two tiny AllReduces for BatchNorm statistics.
"""
import numpy as np
import ml_dtypes
import concourse.bass as bass
import concourse.bacc as bacc
import concourse.mybir as mybir
from concourse.tile import TileContext
from concourse.bass_utils import run_bass_kernel_spmd

F32 = mybir.dt.float32
F32R = mybir.dt.float32r
BF16 = mybir.dt.bfloat16
I16 = mybir.dt.int16
AF = mybir.ActivationFunctionType
ALU = mybir.AluOpType

NCORES = 8
CALLCH = 12          # gather-call granularity in 128-slot chunks

# --- queue-aware DMASW semaphore lane assignment -------------------------
# Tile rotates Pool-engine DMA completion sems over 8 lanes blindly; with
# multiple SWDGE queues a lane must stay bound to one queue (completions
# are only ordered within a queue). Give each queue a dedicated lane pair.
import concourse.tile_sem_assignment as _tsa

if not getattr(_tsa, "_qaware_patched", False):
    _orig_assign_tick = _tsa.TileClockTick._assign_tick

    def _assign_tick_qaware(self, inst):
        if (isinstance(inst, _tsa.DMAInst)
                and inst.engine == mybir.EngineType.Pool
                and not isinstance(inst, _tsa.bass_isa.UserSyncedRemoteDMADescs)):
            q = int(getattr(inst, "queue_num", 0) or 0)
            tog = getattr(self, "_q_tog", None)
            if tog is None:
                tog = self._q_tog = {}
            self.next_sw_dma_idx = q * 2 + tog.get(q, 0)
            tog[q] = 1 - tog.get(q, 0)
        return _orig_assign_tick(self, inst)

    _tsa.TileClockTick._assign_tick = _assign_tick_qaware
    _tsa._qaware_patched = True


def _full_cfg():
    return dict(N=65536, G=128, L=512, D=128, E=262144, STEPS=6)


# --------------------------------------------------------------------------
# host-side edge preprocessing
# --------------------------------------------------------------------------

def _prep_edges(cfg, edge_index, edge_weight):
    N, E = cfg["N"], cfg["E"]
    SH = N // NCORES
    NBLK = SH // 256
    src = np.asarray(edge_index[0], dtype=np.int64)
    dst = np.asarray(edge_index[1], dtype=np.int64)
    w = np.asarray(edge_weight, dtype=np.float32)

    per_core = []
    counts = np.zeros((NCORES, NBLK, 2), dtype=np.int64)
    for c in range(NCORES):
        m = (dst // SH) == c
        s, d, ww = src[m], dst[m] - c * SH, w[m]
        blk = d >> 8
        din = d & 255
        # table halves = row ranges of the single AllGather output (cores 0-3 | 4-7)
        half = (s // (N // 2)).astype(np.int64)
        row = (s % (N // 2))
        order = np.lexsort((half, blk))
        per_core.append((row[order], din[order], ww[order], blk[order], half[order]))
        np.add.at(counts[c], (blk, half), 1)

    # common chunk layout: per (block, parity) the max chunk count over cores
    nch = np.ceil(counts / 128.0).astype(np.int64).max(axis=0)
    for b in range(NBLK):
        if nch[b].sum() == 0:
            nch[b, 0] = 1
    chunks = []   # (block, half), half-major so half-0 gathers can chase AG1
    for p in range(2):
        for b in range(NBLK):
            for _ in range(int(nch[b, p])):
                chunks.append((b, p))
    TOTCH = len(chunks)
    TOT = TOTCH * 128

    gidx_all, ind_all = [], []
    for c in range(NCORES):
        s, din, ww, blk, par = per_core[c]
        idx_sl = np.zeros(TOT, dtype=np.int16)
        w_sl = np.zeros(TOT, dtype=np.float32)
        d_sl = np.zeros(TOT, dtype=np.int64)
        cc = np.zeros((NBLK, 2), dtype=np.int64)
        np.add.at(cc, (blk, par), 1)
        starts = {}
        off = 0
        for b in range(NBLK):
            for p in range(2):
                starts[(b, p)] = off
                off += cc[b, p]
        used = {k: 0 for k in starts}
        pos = 0
        for (b, p) in chunks:
            st = starts[(b, p)] + used[(b, p)]
            n = int(min(128, cc[b, p] - used[(b, p)]))
            if n > 0:
                sl = slice(st, st + n)
                idx_sl[pos:pos + n] = s[sl].astype(np.int16)
                w_sl[pos:pos + n] = ww[sl]
                d_sl[pos:pos + n] = din[sl]
                used[(b, p)] += n
            pos += 128
        ind = np.zeros((TOT, 256), dtype=np.float32)
        ind[np.arange(TOT), d_sl] = w_sl
        # [(c e), d] -> [e, (c d)] so each gather-call's slice is contiguous per partition
        indT = np.ascontiguousarray(
            ind.reshape(-1, 128, 256).transpose(1, 0, 2).reshape(128, -1))
        ind_all.append(indT.astype(ml_dtypes.bfloat16))
        gi = np.tile(idx_sl.reshape(TOT // 16, 16).T, (8, 1)).copy()
        gidx_all.append(gi)

    calls = []
    h1 = next((i for i, (b, p) in enumerate(chunks) if p == 1), TOTCH)
    for lo, hi in ((0, h1), (h1, TOTCH)):
        ch0 = lo
        while ch0 < hi:
            n = min(CALLCH, hi - ch0)
            calls.append((ch0, n))
            ch0 += n
    return dict(chunks=chunks, calls=calls, TOTCH=TOTCH,
                gidx=gidx_all, ind=ind_all, NBLK=NBLK)


# --------------------------------------------------------------------------
# kernel builder (one SPMD program)
# --------------------------------------------------------------------------

def _build(cfg, lay):
    N, G, L, D, STEPS = cfg["N"], cfg["G"], cfg["L"], cfg["D"], cfg["STEPS"]
    SH = N // NCORES
    GPC = G // NCORES          # graphs per core
    NBLK = lay["NBLK"]
    NT = SH // 512             # 512-node tiles per core
    MCH = SH // 128            # m-matmul chunks
    TOTCH = lay["TOTCH"]
    chunks, calls = lay["chunks"], lay["calls"]
    Lp = L - 2                 # 510
    P1 = (Lp - 3) // 2 + 1     # 254
    L4 = (P1 - 2) // 2 + 1     # 127
    NN1 = float(G * Lp)
    NN2 = float(G * P1)

    nc = bacc.Bacc(None, target_bir_lowering=False, debug=False,
                   num_swdge_queues=4)

    # ---- I/O ----
    xT_in = nc.declare_dram_parameter("xT", [128, SH], F32, isOutput=False)
    gidx_in = nc.declare_dram_parameter("gidx", [128, TOTCH * 8], I16, isOutput=False)
    ind_in = nc.declare_dram_parameter("ind", [128, TOTCH * 256], BF16, isOutput=False)
    wgg_in = nc.declare_dram_parameter("wgg", [STEPS, 128, 128], F32, isOutput=False)
    wih_in = nc.declare_dram_parameter("wihT", [128, 384], F32, isOutput=False)
    whh_in = nc.declare_dram_parameter("whhT", [128, 384], F32, isOutput=False)
    gb_in = nc.declare_dram_parameter("gbias", [128, 4], F32, isOutput=False)
    c1w_in = nc.declare_dram_parameter("c1w", [3, 128, 128], F32, isOutput=False)
    c2w_in = nc.declare_dram_parameter("c2w", [128, 128], F32, isOutput=False)
    cc1w_in = nc.declare_dram_parameter("cc1w", [12, 128, 128], F32, isOutput=False)
    cc2w_in = nc.declare_dram_parameter("cc2w", [4, 128, 128], F32, isOutput=False)
    bn_in = nc.declare_dram_parameter("bnp", [128, 6], F32, isOutput=False)
    mlpy_in = nc.declare_dram_parameter("mlpyT", [128, 2], F32, isOutput=False)
    mlpz_in = nc.declare_dram_parameter("mlpzT", [128, 4], F32, isOutput=False)
    mlpb_in = nc.declare_dram_parameter("mlpb", [2, 2], F32, isOutput=False)
    out_p = nc.declare_dram_parameter("out", [GPC, 2], F32, isOutput=True)

    # ---- internal DRAM ----
    m_loc = [nc.dram_tensor(f"m_loc{i}", [SH, D], BF16) for i in range(2)]
    m_full = [nc.dram_tensor(f"m_full{i}", [N, D], BF16) for i in range(2)]
    ar1_in = nc.dram_tensor("ar1_in", [128, 6], F32)
    ar1_out = nc.dram_tensor("ar1_out", [128, 6], F32)
    ar2_in = nc.dram_tensor("ar2_in", [128, 6], F32)
    ar2_out = nc.dram_tensor("ar2_out", [128, 6], F32)

    rg = [list(range(NCORES))]

    with TileContext(nc) as tc:
      with tc.tile_pool(name="persist", bufs=1) as pp:
        hT = pp.tile([128, SH], F32R)
        xT = pp.tile([128, SH], F32R)
        nc.gpsimd.dma_start(out=hT[:], in_=xT_in[:, :])
        nc.gpsimd.dma_start(out=xT[:], in_=xT_in[:, :])

        # ================= GGNN =================
        with tc.tile_pool(name="ggnn_sb", bufs=1) as gsb, \
             tc.tile_pool(name="gath", bufs=4) as gpool, \
             tc.tile_pool(name="indp", bufs=3) as ipool, \
             tc.tile_pool(name="psA", bufs=2, space="PSUM") as psA, \
             tc.tile_pool(name="psB", bufs=1, space="PSUM") as psB:

            aggT = gsb.tile([128, SH], F32R)
            idx_t = gsb.tile([128, TOTCH * 8], I16)
            nc.sync.dma_start(out=idx_t[:], in_=gidx_in[:, :])
            wih = gsb.tile([128, 384], F32R)
            nc.gpsimd.dma_start(out=wih[:], in_=wih_in[:, :])
            whh = gsb.tile([128, 384], F32R)
            nc.gpsimd.dma_start(out=whh[:], in_=whh_in[:, :])
            wgg = gsb.tile([128, STEPS * 128], F32R)
            nc.gpsimd.dma_start(out=wgg[:].rearrange("k (s d) -> k s d", d=128), in_=wgg_in.rearrange("s k d -> k s d"))
            gbias = gsb.tile([128, 4], F32)
            nc.sync.dma_start(out=gbias[:], in_=gb_in[:, :])

            ph_first = {}
            ph_last = {}
            for t, (b, p) in enumerate(chunks):
                ph_first.setdefault((b, p), t)
                ph_last[(b, p)] = t
            first_phase = {b: p for (b, p) in sorted(ph_first, reverse=True)}

            for s in range(STEPS):
                with nc.named_scope(f"step{s}"):
                    # ---- m = h @ W[s]  (natural layout, bf16) ----
                    m_stage = gsb.tile([128, SH], BF16, tag="m_stage", name="m_stage")
                    for mg in range(MCH // 4):
                        mps = psA.tile([128, 512], F32, tag="mps", name="mps")
                        for j in range(4):
                            n = mg * 4 + j
                            nc.tensor.matmul(
                                mps[:, j * 128:(j + 1) * 128],
                                hT[:, n * 128:(n + 1) * 128],
                                wgg[:, s * 128:(s + 1) * 128],
                                start=True, stop=True)
                        nc.vector.tensor_copy(out=m_stage[:, mg * 512:(mg + 1) * 512], in_=mps[:])
                    B = s % 2
                    mlv = m_loc[B].rearrange("(n p) d -> p n d", p=128)
                    msv = m_stage[:].rearrange("p (n d) -> p n d", d=128)
                    nc.sync.dma_start(out=mlv[:, :MCH // 2, :], in_=msv[:, :MCH // 2, :])
                    nc.sync.dma_start(out=mlv[:, MCH // 2:, :], in_=msv[:, MCH // 2:, :])

                    # ---- AllGather bf16 message table (two halves) ----
                    nc.gpsimd.collective_compute(
                        "AllGather", ALU.bypass, replica_groups=rg,
                        ins=[m_loc[B][:, :]], outs=[m_full[B][:, :]])

                    # ---- gather + PE scatter into aggT ----
                    grp_ps = {}
                    for ci, (c0, ncall) in enumerate(calls):
                        half = chunks[c0][1]
                        tabl = m_full[B][:N // 2, :] if half == 0 else m_full[B][N // 2:, :]
                        gt = gpool.tile([128, CALLCH, 128], BF16, tag="gt", name="gt")
                        nc.gpsimd.dma_gather(
                            out_ap=gt[:, :ncall, :],
                            in_ap=tabl[:, :],
                            idxs_ap=idx_t[:, c0 * 8:(c0 + ncall) * 8],
                            num_idxs=ncall * 128,
                            num_idxs_reg=ncall * 128,
                            elem_size=128,
                            single_packet=False,
                            queue_num=ci % 4,
                        )
                        it = ipool.tile([128, CALLCH, 256], BF16, tag="it", name="it")
                        nc.sync.dma_start(
                            out=it[:, :ncall, :],
                            in_=ind_in[:, c0 * 256:(c0 + ncall) * 256])
                        for j in range(ncall):
                            t = c0 + j
                            b, p = chunks[t]
                            g = (b // 2, p)
                            if g not in grp_ps:
                                grp_ps[g] = psA.tile([128, 512], F32, tag="aggps", name="aggps")
                            off = (b % 2) * 256
                            nc.tensor.matmul(
                                grp_ps[g][:, off:off + 256],
                                gt[:, j, :],
                                it[:, j, :],
                                start=(t == ph_first[(b, p)]),
                                stop=(t == ph_last[(b, p)]))
                            if t == ph_last[(b, p)]:
                                asl = slice(b * 256, (b + 1) * 256)
                                psl = grp_ps[g][:, off:off + 256]
                                if p == first_phase[b]:
                                    nc.vector.tensor_copy(out=aggT[:, asl], in_=psl)
                                else:
                                    nc.vector.tensor_add(out=aggT[:, asl], in0=aggT[:, asl], in1=psl)
                                if b % 2 == 1 or b == NBLK - 1:
                                    del grp_ps[g]

                    # ---- GRU over 512-node tiles ----
                    for t in range(NT):
                        sl = slice(t * 512, (t + 1) * 512)
                        r_ps = psB.tile([128, 512], F32, tag="rps", name="r_ps")
                        z_ps = psB.tile([128, 512], F32, tag="zps", name="z_ps")
                        xn_ps = psB.tile([128, 512], F32, tag="xnps", name="xn_ps")
                        hn_ps = psB.tile([128, 512], F32, tag="hnps", name="hn_ps")
                        nc.tensor.matmul(r_ps[:], wih[:, 0:128], aggT[:, sl], start=True, stop=False)
                        nc.tensor.matmul(r_ps[:], whh[:, 0:128], hT[:, sl], start=False, stop=True)
                        nc.tensor.matmul(z_ps[:], wih[:, 128:256], aggT[:, sl], start=True, stop=False)
                        nc.tensor.matmul(z_ps[:], whh[:, 128:256], hT[:, sl], start=False, stop=True)
                        nc.tensor.matmul(xn_ps[:], wih[:, 256:384], aggT[:, sl], start=True, stop=True)
                        nc.tensor.matmul(hn_ps[:], whh[:, 256:384], hT[:, sl], start=True, stop=True)

                        r_sb = gsb.tile([128, 512], F32, tag="r_sb", name="r_sb")
                        z_sb = gsb.tile([128, 512], F32, tag="z_sb", name="z_sb")
                        nc.scalar.activation(out=r_sb[:], in_=r_ps[:], func=AF.Sigmoid, bias=gbias[:, 0:1])
                        nc.scalar.activation(out=z_sb[:], in_=z_ps[:], func=AF.Sigmoid, bias=gbias[:, 1:2])
                        t1 = gsb.tile([128, 512], F32, tag="t1", name="t1")
                        nc.vector.tensor_mul(out=t1[:], in0=r_sb[:], in1=hn_ps[:])
                        t2 = gsb.tile([128, 512], F32, tag="t2", name="t2")
                        nc.vector.tensor_add(out=t2[:], in0=t1[:], in1=xn_ps[:])
                        n_sb = gsb.tile([128, 512], F32, tag="n_sb", name="n_sb")
                        nc.scalar.activation(out=n_sb[:], in_=t2[:], func=AF.Tanh, bias=gbias[:, 2:3])
                        d_sb = gsb.tile([128, 512], F32, tag="d_sb", name="d_sb")
                        nc.vector.tensor_sub(out=d_sb[:], in0=hT[:, sl], in1=n_sb[:])
                        zd = gsb.tile([128, 512], F32, tag="zd", name="zd")
                        nc.vector.tensor_mul(out=zd[:], in0=z_sb[:], in1=d_sb[:])
                        nc.vector.tensor_add(out=hT[:, sl], in0=n_sb[:], in1=zd[:])

        # ================= conv/MLP head =================
        with tc.tile_pool(name="head_sb", bufs=1) as hsb:

            bnp = hsb.tile([128, 6], F32)
            nc.sync.dma_start(out=bnp[:], in_=bn_in[:, :])
            st1 = hsb.tile([128, 6], F32)
            nc.vector.memset(st1[:], 0.0)
            st2 = hsb.tile([128, 6], F32)
            nc.vector.memset(st2[:], 0.0)
            sq = hsb.tile([128, 516], F32)
            relu_t = hsb.tile([128, 512], F32)
            y2 = hsb.tile([128, GPC * 256], F32R)
            z2a = hsb.tile([128, GPC * 256], F32R)
            z2b = hsb.tile([128, GPC * 256], F32R)
            ab1 = hsb.tile([128, 6], F32)
            ab2 = hsb.tile([128, 6], F32)

            def stats_into(ps_t, cols, col, n):
                nc.vector.reduce_sum(out=sq[:, 0:1], in_=ps_t[:, :n], axis=mybir.AxisListType.X)
                nc.vector.tensor_add(out=cols[:, col:col + 1], in0=cols[:, col:col + 1], in1=sq[:, 0:1])
                nc.scalar.activation(out=sq[:, 2:2 + n], in_=ps_t[:, :n], func=AF.Square)
                nc.vector.reduce_sum(out=sq[:, 1:2], in_=sq[:, 2:2 + n], axis=mybir.AxisListType.X)
                nc.vector.tensor_add(out=cols[:, col + 1:col + 2], in0=cols[:, col + 1:col + 2], in1=sq[:, 1:2])

            def bn_coeffs(st, col, g_col, b_col, nn, ab, acol):
                mean = hsb.tile([128, 1], F32, tag="bnm", name="bnm")
                nc.vector.tensor_scalar_mul(mean[:], st[:, col:col + 1], 1.0 / nn)
                var = hsb.tile([128, 1], F32, tag="bnv", name="bnv")
                nc.vector.tensor_scalar_mul(var[:], st[:, col + 1:col + 2], 1.0 / nn)
                msq = hsb.tile([128, 1], F32, tag="bnq", name="bnq")
                nc.vector.tensor_mul(out=msq[:], in0=mean[:], in1=mean[:])
                nc.vector.tensor_sub(out=var[:], in0=var[:], in1=msq[:])
                nc.vector.tensor_scalar_add(var[:], var[:], 1e-5)
                sd = hsb.tile([128, 1], F32, tag="bnsd", name="bnsd")
                nc.scalar.activation(out=sd[:], in_=var[:], func=AF.Sqrt)
                inv = hsb.tile([128, 1], F32, tag="bninv", name="bninv")
                nc.vector.reciprocal(out=inv[:], in_=sd[:])
                nc.vector.tensor_mul(out=ab[:, acol:acol + 1], in0=inv[:], in1=bnp[:, g_col:g_col + 1])
                nc.vector.tensor_mul(out=mean[:], in0=mean[:], in1=ab[:, acol:acol + 1])
                nc.vector.tensor_sub(out=ab[:, acol + 1:acol + 2], in0=bnp[:, b_col:b_col + 1], in1=mean[:])

            def bn_relu_pool3(src_ap, acol, out_ap, ab):
                # bn+relu then maxpool k=3 s=2: [*, Lp] -> [*, P1]
                nc.scalar.activation(out=relu_t[:, :Lp], in_=src_ap, func=AF.Relu,
                                     bias=ab[:, acol + 1:acol + 2], scale=ab[:, acol:acol + 1])
                a = relu_t[:, 0:2 * P1].rearrange("p (l t) -> p t l", t=2)
                bb = relu_t[:, 2:2 + 2 * P1].rearrange("p (l t) -> p t l", t=2)
                mx = hsb.tile([128, P1], F32, tag="mx", name="mx")
                nc.vector.tensor_max(out=mx[:], in0=a[:, 0, :], in1=a[:, 1, :])
                nc.vector.tensor_max(out=out_ap, in0=mx[:], in1=bb[:, 0, :])

            # ---- phase A/B: conv1+convc1, stats, bn+relu+pool ----
            with tc.tile_pool(name="pA_sb", bufs=1) as pa, \
                 tc.tile_pool(name="pA_ps", bufs=2, space="PSUM") as hps:
                c1w = pa.tile([128, 3 * 128], F32R)
                nc.gpsimd.dma_start(out=c1w[:].rearrange("a (k b) -> a k b", b=128), in_=c1w_in.rearrange("k a b -> a k b"))
                cc1w = pa.tile([128, 12 * 128], F32R)
                nc.gpsimd.dma_start(out=cc1w[:].rearrange("a (k b) -> a k b", b=128), in_=cc1w_in.rearrange("k a b -> a k b"))
                y1 = pa.tile([128, GPC * 512], BF16)
                z1a = pa.tile([128, GPC * 512], BF16)
                z1b = pa.tile([128, GPC * 512], BF16)

                for g in range(GPC):
                    gs = slice(g * 512, g * 512 + 512)
                    hg = hT[:, gs]
                    xg = xT[:, gs]
                    c1ps = hps.tile([128, 512], F32, tag="c1ps", name="c1ps")
                    for k in range(3):
                        nc.tensor.matmul(c1ps[:, :Lp], c1w[:, k * 128:(k + 1) * 128],
                                         hg[:, k:k + Lp], start=(k == 0), stop=(k == 2))
                    stats_into(c1ps, st1, 0, Lp)
                    nc.vector.tensor_copy(out=y1[:, g * 512:g * 512 + Lp], in_=c1ps[:, :Lp])
                    for co in range(2):
                        ccps = hps.tile([128, 512], F32, tag="ccps", name="ccps")
                        for k in range(3):
                            nc.tensor.matmul(ccps[:, :Lp],
                                             cc1w[:, (k * 4 + co) * 128:(k * 4 + co) * 128 + 128],
                                             hg[:, k:k + Lp], start=(k == 0), stop=False)
                        for k in range(3):
                            nc.tensor.matmul(ccps[:, :Lp],
                                             cc1w[:, (k * 4 + 2 + co) * 128:(k * 4 + 2 + co) * 128 + 128],
                                             xg[:, k:k + Lp], start=False, stop=(k == 2))
                        stats_into(ccps, st1, 2 + 2 * co, Lp)
                        dst = z1a if co == 0 else z1b
                        nc.vector.tensor_copy(out=dst[:, g * 512:g * 512 + Lp], in_=ccps[:, :Lp])

                nc.sync.dma_start(out=ar1_in[:, :], in_=st1[:])
                nc.gpsimd.collective_compute("AllReduce", ALU.add, replica_groups=rg,
                                             ins=[ar1_in[:, :]], outs=[ar1_out[:, :]])
                nc.sync.dma_start(out=st1[:], in_=ar1_out[:, :])
                bn_coeffs(st1, 0, 0, 1, NN1, ab1, 0)
                bn_coeffs(st1, 2, 2, 3, NN1, ab1, 2)
                bn_coeffs(st1, 4, 4, 5, NN1, ab1, 4)

                nc.vector.memset(y2[:].bitcast(F32), 0.0)
                nc.vector.memset(z2a[:].bitcast(F32), 0.0)
                nc.vector.memset(z2b[:].bitcast(F32), 0.0)
                for g in range(GPC):
                    gs = slice(g * 512, g * 512 + 512)
                    o = g * 256
                    bn_relu_pool3(y1[:, gs][:, :Lp], 0, y2[:, o:o + P1], ab1)
                    bn_relu_pool3(z1a[:, gs][:, :Lp], 2, z2a[:, o:o + P1], ab1)
                    bn_relu_pool3(z1b[:, gs][:, :Lp], 4, z2b[:, o:o + P1], ab1)

            # ---- phase C: conv2/convc2 + stats2 + bn/relu/pool + proj ----
            with tc.tile_pool(name="pC_sb", bufs=1) as pc, \
                 tc.tile_pool(name="pC_ps", bufs=2, space="PSUM") as hps:
                c2w = pc.tile([128, 128], F32R)
                nc.gpsimd.dma_start(out=c2w[:], in_=c2w_in[:, :])
                cc2w = pc.tile([128, 4 * 128], F32R)
                nc.gpsimd.dma_start(out=cc2w[:].rearrange("a (k b) -> a k b", b=128), in_=cc2w_in.rearrange("k a b -> a k b"))
                y3 = pc.tile([128, GPC * 256], F32)
                z3a = pc.tile([128, GPC * 256], F32)
                z3b = pc.tile([128, GPC * 256], F32)

                for g in range(GPC):
                    gs = slice(g * 256, g * 256 + 256)
                    c2ps = hps.tile([128, 256], F32, tag="c2ps", name="c2ps")
                    nc.tensor.matmul(c2ps[:], c2w[:], y2[:, gs], start=True, stop=True)
                    stats_into(c2ps, st2, 0, P1)
                    nc.vector.tensor_copy(out=y3[:, gs], in_=c2ps[:])
                    for co in range(2):
                        ccps2 = hps.tile([128, 256], F32, tag="ccps2", name="ccps2")
                        nc.tensor.matmul(ccps2[:], cc2w[:, co * 128:co * 128 + 128],
                                         z2a[:, gs], start=True, stop=False)
                        nc.tensor.matmul(ccps2[:], cc2w[:, (2 + co) * 128:(2 + co) * 128 + 128],
                                         z2b[:, gs], start=False, stop=True)
                        stats_into(ccps2, st2, 2 + 2 * co, P1)
                        nc.vector.tensor_copy(out=(z3a if co == 0 else z3b)[:, gs], in_=ccps2[:])

                nc.sync.dma_start(out=ar2_in[:, :], in_=st2[:])
                nc.gpsimd.collective_compute("AllReduce", ALU.add, replica_groups=rg,
                                             ins=[ar2_in[:, :]], outs=[ar2_out[:, :]])
                nc.sync.dma_start(out=st2[:], in_=ar2_out[:, :])
                bn_coeffs(st2, 0, 0, 1, NN2, ab2, 0)
                bn_coeffs(st2, 2, 2, 3, NN2, ab2, 2)
                bn_coeffs(st2, 4, 4, 5, NN2, ab2, 4)

                mlpy = pc.tile([128, 2], F32)
                nc.sync.dma_start(out=mlpy[:], in_=mlpy_in[:, :])
                mlpz = pc.tile([128, 4], F32)
                nc.sync.dma_start(out=mlpz[:], in_=mlpz_in[:, :])
                mlpb = pc.tile([2, 2], F32)
                nc.sync.dma_start(out=mlpb[:], in_=mlpb_in[:, :])
                outsb = pc.tile([2, GPC], F32)
                y4 = pc.tile([128, 128], F32)
                z4a = pc.tile([128, 128], F32)
                z4b = pc.tile([128, 128], F32)

                def bn_relu_pool2(src_t, gs, acol, out_t, ab):
                    nc.scalar.activation(out=relu_t[:, :P1], in_=src_t[:, gs][:, :P1], func=AF.Relu,
                                         bias=ab[:, acol + 1:acol + 2], scale=ab[:, acol:acol + 1])
                    a = relu_t[:, 0:2 * L4].rearrange("p (l t) -> p t l", t=2)
                    nc.vector.tensor_max(out=out_t[:, :L4], in0=a[:, 0, :], in1=a[:, 1, :])

                for g in range(GPC):
                    gs = slice(g * 256, g * 256 + 256)
                    bn_relu_pool2(y3, gs, 0, y4, ab2)
                    bn_relu_pool2(z3a, gs, 2, z4a, ab2)
                    bn_relu_pool2(z3b, gs, 4, z4b, ab2)
                    yp = hps.tile([2, L4], F32, tag="yp", name="yp")
                    nc.tensor.matmul(yp[:], mlpy[:], y4[:, :L4], start=True, stop=True)
                    zp = hps.tile([2, L4], F32, tag="zp", name="zp")
                    nc.tensor.matmul(zp[:], mlpz[:, 0:2], z4a[:, :L4], start=True, stop=False)
                    nc.tensor.matmul(zp[:], mlpz[:, 2:4], z4b[:, :L4], start=False, stop=True)
                    ypb = pc.tile([2, L4], F32, tag="ypb", name="ypb")
                    nc.vector.tensor_scalar_add(ypb[:], yp[:], mlpb[:, 0:1])
                    zpb = pc.tile([2, L4], F32, tag="zpb", name="zpb")
                    nc.vector.tensor_scalar_add(zpb[:], zp[:], mlpb[:, 1:2])
                    prod = pc.tile([2, L4], F32, tag="prod", name="prod")
                    nc.vector.tensor_mul(out=prod[:], in0=ypb[:], in1=zpb[:])
                    nc.vector.reduce_sum(out=outsb[:, g:g + 1], in_=prod[:], axis=mybir.AxisListType.X)
                nc.vector.tensor_scalar_mul(outsb[:], outsb[:], 1.0 / L4)
                nc.sync.dma_start(out=out_p.rearrange("g p -> p g"), in_=outsb[:])

    nc.finalize()
    return nc


# --------------------------------------------------------------------------
# host weight packing
# --------------------------------------------------------------------------

def _make_inmaps(cfg, lay, inputs):
    N = cfg["N"]
    SH = N // NCORES
    f32 = np.float32
    x = np.asarray(inputs["x"], f32)
    wgg = np.ascontiguousarray(np.asarray(inputs["ggnn_w"], f32))
    wihT = np.ascontiguousarray(np.asarray(inputs["gru_wih"], f32).T)
    whhT = np.ascontiguousarray(np.asarray(inputs["gru_whh"], f32).T)
    bih = np.asarray(inputs["gru_bih"], f32)
    bhh = np.asarray(inputs["gru_bhh"], f32)
    gbias = np.zeros((128, 4), f32)
    gbias[:, 0] = bih[0:128] + bhh[0:128]
    gbias[:, 1] = bih[128:256] + bhh[128:256]
    gbias[:, 2] = bih[256:384]
    gbias[:, 3] = bhh[256:384]
    assert np.all(bhh[256:384] == 0), "nonzero bhh_n not supported"

    c1 = np.asarray(inputs["conv1_w"], f32)
    c1w = np.ascontiguousarray(np.transpose(c1, (2, 1, 0)))
    c2w = np.ascontiguousarray(np.asarray(inputs["conv2_w"], f32)[:, :, 0].T)
    cc1 = np.asarray(inputs["convc1_w"], f32)
    cc1w = np.zeros((12, 128, 128), f32)
    for k in range(3):
        for ci in range(2):
            for co in range(2):
                cc1w[k * 4 + ci * 2 + co] = cc1[co * 128:(co + 1) * 128,
                                                ci * 128:(ci + 1) * 128, k].T
    cc2 = np.asarray(inputs["convc2_w"], f32)[:, :, 0]
    cc2w = np.zeros((4, 128, 128), f32)
    for ci in range(2):
        for co in range(2):
            cc2w[ci * 2 + co] = cc2[co * 128:(co + 1) * 128, ci * 128:(ci + 1) * 128].T
    bnp = np.zeros((128, 6), f32)
    bnp[:, 0] = np.asarray(inputs["bn1_g"], f32)
    bnp[:, 1] = np.asarray(inputs["bn1_b"], f32)
    bn2g = np.asarray(inputs["bn2_g"], f32)
    bn2b = np.asarray(inputs["bn2_b"], f32)
    bnp[:, 2] = bn2g[:128]; bnp[:, 3] = bn2b[:128]
    bnp[:, 4] = bn2g[128:]; bnp[:, 5] = bn2b[128:]
    mlpyT = np.ascontiguousarray(np.asarray(inputs["mlpy_w"], f32).T)
    mzw = np.asarray(inputs["mlpz_w"], f32)
    mlpzT = np.zeros((128, 4), f32)
    mlpzT[:, 0:2] = mzw[:, :128].T
    mlpzT[:, 2:4] = mzw[:, 128:].T
    mlpb = np.zeros((2, 2), f32)
    mlpb[:, 0] = np.asarray(inputs["mlpy_b"], f32)
    mlpb[:, 1] = np.asarray(inputs["mlpz_b"], f32)

    common = dict(wgg=wgg, wihT=wihT, whhT=whhT, gbias=gbias, c1w=c1w, c2w=c2w,
                  cc1w=cc1w, cc2w=cc2w, bnp=bnp, mlpyT=mlpyT, mlpzT=mlpzT, mlpb=mlpb)
    in_maps = []
    for c in range(NCORES):
        xT = np.ascontiguousarray(x[c * SH:(c + 1) * SH].T)
        in_maps.append(dict(xT=xT, gidx=lay["gidx"][c], ind=lay["ind"][c], **common))
    return in_maps


def run(cfg, inputs, trace=False):
    lay = _prep_edges(cfg, inputs["edge_index"], inputs["edge_weight"])
    nc = _build(cfg, lay)
    in_maps = _make_inmaps(cfg, lay, inputs)
    res = run_bass_kernel_spmd(nc, in_maps, list(range(NCORES)), trace=trace)
    out = np.concatenate([res.results[c]["out"] for c in range(NCORES)], axis=0)
    return out.astype(np.float32), res


def kernel(**inputs) -> np.ndarray:
    out, _ = run(_full_cfg(), inputs, trace=False)
    return out



# revision 26
# speedup vs baseline: 1.2132x; 1.2132x over previous
"""DevignModel (GGNN + conv head) Trainium2 Bass kernel, 8-core SPMD.

Sharding: nodes/graphs split 8 ways (8192 nodes = 16 graphs per core).
Pipelined GGNN step: the bf16 message table is AllGathered in TWO halves
(rows = each core's first/second 4096 local nodes) so that half-0 edge
gathers only wait on AG_a. The GRU update and the NEXT step's message
matmul are interleaved into the phase-1 scatter stream tile-by-tile, so
AG_a of step s+1 launches mid-phase-1 of step s and overlaps the gather
tail. Edge aggregation: 4-queue dma_gather of message rows + PE
weighted-indicator matmul into aggT. The conv/BN/MLP head keeps BN
statistics on the ACT accumulator / GpSimd so DVE stops being the
bottleneck, with two tiny AllReduces for cross-core BatchNorm.
"""
import numpy as np
import ml_dtypes
import concourse.bass as bass
import concourse.bacc as bacc
import concourse.mybir as mybir
from concourse.tile import TileContext
from concourse.bass_utils import run_bass_kernel_spmd

F32 = mybir.dt.float32
F32R = mybir.dt.float32r
BF16 = mybir.dt.bfloat16
I16 = mybir.dt.int16
AF = mybir.ActivationFunctionType
ALU = mybir.AluOpType

NCORES = 8
CALLCH = 12          # gather-call granularity in 128-slot chunks

# --- queue-aware DMASW semaphore lane assignment -------------------------
# Tile rotates Pool-engine DMA completion sems over 8 lanes blindly; with
# multiple SWDGE queues a lane must stay bound to one queue (completions
# are only ordered within a queue). Give each queue a dedicated lane pair.
import concourse.tile_sem_assignment as _tsa

if not getattr(_tsa, "_qaware_patched", False):
    _orig_assign_tick = _tsa.TileClockTick._assign_tick

    def _assign_tick_qaware(self, inst):
        if (isinstance(inst, _tsa.DMAInst)
                and inst.engine == mybir.EngineType.Pool
                and not isinstance(inst, _tsa.bass_isa.UserSyncedRemoteDMADescs)):
            q = int(getattr(inst, "queue_num", 0) or 0)
            tog = getattr(self, "_q_tog", None)
            if tog is None:
                tog = self._q_tog = {}
            self.next_sw_dma_idx = q * 2 + tog.get(q, 0)
            tog[q] = 1 - tog.get(q, 0)
        return _orig_assign_tick(self, inst)

    _tsa.TileClockTick._assign_tick = _assign_tick_qaware
    _tsa._qaware_patched = True


def _full_cfg():
    return dict(N=65536, G=128, L=512, D=128, E=262144, STEPS=6)


# --------------------------------------------------------------------------
# host-side edge preprocessing
# --------------------------------------------------------------------------

def _prep_edges(cfg, edge_index, edge_weight):
    N, E = cfg["N"], cfg["E"]
    SH = N // NCORES
    HH = SH // 2
    NBLK = SH // 256
    src = np.asarray(edge_index[0], dtype=np.int64)
    dst = np.asarray(edge_index[1], dtype=np.int64)
    w = np.asarray(edge_weight, dtype=np.float32)

    per_core = []
    counts = np.zeros((NCORES, NBLK, 2), dtype=np.int64)
    for c in range(NCORES):
        m = (dst // SH) == c
        s, d, ww = src[m], dst[m] - c * SH, w[m]
        blk = d >> 8
        din = d & 255
        # table halves = first/second 4096 local nodes of each source core;
        # AllGather of those slices puts global node g at row
        # (g // SH) * HH + (g % HH) of its half's table (int16-safe).
        half = ((s % SH) // HH).astype(np.int64)
        row = (s // SH) * HH + (s % HH)
        order = np.lexsort((half, blk))
        per_core.append((row[order], din[order], ww[order], blk[order], half[order]))
        np.add.at(counts[c], (blk, half), 1)

    # common chunk layout: per (block, phase) the max chunk count over cores
    nch = np.ceil(counts / 128.0).astype(np.int64).max(axis=0)
    for b in range(NBLK):
        if nch[b].sum() == 0:
            nch[b, 0] = 1
    chunks = []   # phase-major (half 0 chunks first), block-ascending inside
    for p in range(2):
        for b in range(NBLK):
            for _ in range(int(nch[b, p])):
                chunks.append((b, p))
    TOTCH = len(chunks)
    TOT = TOTCH * 128

    gidx_all, ind_all = [], []
    for c in range(NCORES):
        s, din, ww, blk, par = per_core[c]
        idx_sl = np.zeros(TOT, dtype=np.int16)
        w_sl = np.zeros(TOT, dtype=np.float32)
        d_sl = np.zeros(TOT, dtype=np.int64)
        cc = np.zeros((NBLK, 2), dtype=np.int64)
        np.add.at(cc, (blk, par), 1)
        starts = {}
        off = 0
        for b in range(NBLK):
            for p in range(2):
                starts[(b, p)] = off
                off += cc[b, p]
        used = {k: 0 for k in starts}
        pos = 0
        for (b, p) in chunks:
            st = starts[(b, p)] + used[(b, p)]
            n = int(min(128, cc[b, p] - used[(b, p)]))
            if n > 0:
                sl = slice(st, st + n)
                idx_sl[pos:pos + n] = s[sl].astype(np.int16)
                w_sl[pos:pos + n] = ww[sl]
                d_sl[pos:pos + n] = din[sl]
                used[(b, p)] += n
            pos += 128
        ind = np.zeros((TOT, 256), dtype=np.float32)
        ind[np.arange(TOT), d_sl] = w_sl
        # [(c e), d] -> [e, (c d)] so each gather-call's slice is contiguous per partition
        indT = np.ascontiguousarray(
            ind.reshape(-1, 128, 256).transpose(1, 0, 2).reshape(128, -1))
        ind_all.append(indT.astype(ml_dtypes.bfloat16))
        gi = np.tile(idx_sl.reshape(TOT // 16, 16).T, (8, 1)).copy()
        gidx_all.append(gi)

    calls = []
    h1 = next((i for i, (b, p) in enumerate(chunks) if p == 1), TOTCH)
    for lo, hi in ((0, h1), (h1, TOTCH)):
        ch0 = lo
        while ch0 < hi:
            n = min(CALLCH, hi - ch0)
            calls.append((ch0, n))
            ch0 += n
    return dict(chunks=chunks, calls=calls, TOTCH=TOTCH,
                gidx=gidx_all, ind=ind_all, NBLK=NBLK)


# --------------------------------------------------------------------------
# kernel builder (one SPMD program)
# --------------------------------------------------------------------------

def _build(cfg, lay):
    N, G, L, D, STEPS = cfg["N"], cfg["G"], cfg["L"], cfg["D"], cfg["STEPS"]
    SH = N // NCORES
    HH = SH // 2
    GPC = G // NCORES          # graphs per core
    NBLK = lay["NBLK"]
    NT = SH // 512             # 512-node tiles per core
    TOTCH = lay["TOTCH"]
    chunks, calls = lay["chunks"], lay["calls"]
    Lp = L - 2                 # 510
    P1 = (Lp - 3) // 2 + 1     # 254
    L4 = (P1 - 2) // 2 + 1     # 127
    NN1 = float(G * Lp)
    NN2 = float(G * P1)

    nc = bacc.Bacc(None, target_bir_lowering=False, debug=False,
                   num_swdge_queues=4)

    # ---- I/O ----
    xT_in = nc.declare_dram_parameter("xT", [128, SH], F32, isOutput=False)
    gidx_in = nc.declare_dram_parameter("gidx", [128, TOTCH * 8], I16, isOutput=False)
    ind_in = nc.declare_dram_parameter("ind", [128, TOTCH * 256], BF16, isOutput=False)
    wgg_in = nc.declare_dram_parameter("wgg", [STEPS, 128, 128], F32, isOutput=False)
    wih_in = nc.declare_dram_parameter("wihT", [128, 384], F32, isOutput=False)
    whh_in = nc.declare_dram_parameter("whhT", [128, 384], F32, isOutput=False)
    gb_in = nc.declare_dram_parameter("gbias", [128, 4], F32, isOutput=False)
    c1w_in = nc.declare_dram_parameter("c1w", [3, 128, 128], F32, isOutput=False)
    c2w_in = nc.declare_dram_parameter("c2w", [128, 128], F32, isOutput=False)
    cc1w_in = nc.declare_dram_parameter("cc1w", [12, 128, 128], F32, isOutput=False)
    cc2w_in = nc.declare_dram_parameter("cc2w", [4, 128, 128], F32, isOutput=False)
    bn_in = nc.declare_dram_parameter("bnp", [128, 6], F32, isOutput=False)
    mlpy_in = nc.declare_dram_parameter("mlpyT", [128, 2], F32, isOutput=False)
    mlpz_in = nc.declare_dram_parameter("mlpzT", [128, 4], F32, isOutput=False)
    mlpb_in = nc.declare_dram_parameter("mlpb", [2, 2], F32, isOutput=False)
    out_p = nc.declare_dram_parameter("out", [GPC, 2], F32, isOutput=True)

    # ---- internal DRAM ----
    m_loc = [nc.dram_tensor(f"m_loc{i}", [SH, D], BF16) for i in range(2)]
    m_fa = [nc.dram_tensor(f"m_fa{i}", [N // 2, D], BF16, addr_space="Shared")
            for i in range(2)]
    m_fb = [nc.dram_tensor(f"m_fb{i}", [N // 2, D], BF16, addr_space="Shared")
            for i in range(2)]
    ar1_in = nc.dram_tensor("ar1_in", [128, 6], F32)
    ar1_out = nc.dram_tensor("ar1_out", [128, 6], F32)
    ar2_in = nc.dram_tensor("ar2_in", [128, 6], F32)
    ar2_out = nc.dram_tensor("ar2_out", [128, 6], F32)

    rg = [list(range(NCORES))]

    # chunk/phase metadata
    ph_first, ph_last = {}, {}
    for t, (b, p) in enumerate(chunks):
        ph_first.setdefault((b, p), t)
        ph_last[(b, p)] = t
    first_phase, last_phase, fin_chunk = {}, {}, {}
    for b in range(NBLK):
        ps = [p for p in (0, 1) if (b, p) in ph_first]
        first_phase[b] = min(ps)
        last_phase[b] = max(ps)
        fin_chunk[b] = ph_last[(b, last_phase[b])]

    with TileContext(nc) as tc:
      with tc.tile_pool(name="persist", bufs=1) as pp:
        hT = pp.tile([128, SH], F32R)
        xT = pp.tile([128, SH], F32R)
        nc.gpsimd.dma_start(out=hT[:], in_=xT_in[:, :])
        nc.gpsimd.dma_start(out=xT[:], in_=xT_in[:, :])

        # ================= GGNN =================
        with tc.tile_pool(name="ggnn_sb", bufs=1) as gsb, \
             tc.tile_pool(name="gath", bufs=4) as gpool, \
             tc.tile_pool(name="indp", bufs=3) as ipool, \
             tc.tile_pool(name="psA", bufs=2, space="PSUM") as psA, \
             tc.tile_pool(name="psB", bufs=1, space="PSUM") as psB:

            aggT = gsb.tile([128, SH], F32R)
            m_stage = gsb.tile([128, SH], BF16)
            idx_t = gsb.tile([128, TOTCH * 8], I16)
            nc.sync.dma_start(out=idx_t[:], in_=gidx_in[:, :])
            wih = gsb.tile([128, 384], F32R)
            nc.gpsimd.dma_start(out=wih[:], in_=wih_in[:, :])
            whh = gsb.tile([128, 384], F32R)
            nc.gpsimd.dma_start(out=whh[:], in_=whh_in[:, :])
            wgg = gsb.tile([128, STEPS * 128], F32R)
            nc.gpsimd.dma_start(out=wgg[:].rearrange("k (s d) -> k s d", d=128), in_=wgg_in.rearrange("s k d -> k s d"))
            gbias = gsb.tile([128, 4], F32)
            nc.sync.dma_start(out=gbias[:], in_=gb_in[:, :])

            def emit_m_tile(t, s_next, B_next):
                # m[512t : 512t+512] = h @ W[s_next], natural [node, dout] layout
                mps = psA.tile([128, 512], F32, tag="mps", name="mps")
                for j4 in range(4):
                    n = t * 4 + j4
                    nc.tensor.matmul(
                        mps[:, j4 * 128:(j4 + 1) * 128],
                        hT[:, n * 128:(n + 1) * 128],
                        wgg[:, s_next * 128:(s_next + 1) * 128],
                        start=True, stop=True)
                nc.scalar.activation(out=m_stage[:, t * 512:(t + 1) * 512],
                                     in_=mps[:], func=AF.Copy)
                mlv = m_loc[B_next].rearrange("(n p) d -> p n d", p=128)
                msv = m_stage[:].rearrange("p (n d) -> p n d", d=128)
                nc.sync.dma_start(out=mlv[:, t * 4:(t + 1) * 4, :],
                                  in_=msv[:, t * 4:(t + 1) * 4, :])

            def launch_ag_a(B_next):
                nc.gpsimd.collective_compute(
                    "AllGather", ALU.bypass, replica_groups=rg,
                    ins=[m_loc[B_next][:HH, :]], outs=[m_fa[B_next][:, :]])

            def launch_ag_b(B_next):
                nc.gpsimd.collective_compute(
                    "AllGather", ALU.bypass, replica_groups=rg,
                    ins=[m_loc[B_next][HH:, :]], outs=[m_fb[B_next][:, :]])

            def emit_gru_tile(t):
                sl = slice(t * 512, (t + 1) * 512)
                r_ps = psB.tile([128, 512], F32, tag="rps", name="r_ps")
                z_ps = psB.tile([128, 512], F32, tag="zps", name="z_ps")
                xn_ps = psB.tile([128, 512], F32, tag="xnps", name="xn_ps")
                hn_ps = psB.tile([128, 512], F32, tag="hnps", name="hn_ps")
                nc.tensor.matmul(r_ps[:], wih[:, 0:128], aggT[:, sl], start=True, stop=False)
                nc.tensor.matmul(r_ps[:], whh[:, 0:128], hT[:, sl], start=False, stop=True)
                nc.tensor.matmul(z_ps[:], wih[:, 128:256], aggT[:, sl], start=True, stop=False)
                nc.tensor.matmul(z_ps[:], whh[:, 128:256], hT[:, sl], start=False, stop=True)
                nc.tensor.matmul(xn_ps[:], wih[:, 256:384], aggT[:, sl], start=True, stop=True)
                nc.tensor.matmul(hn_ps[:], whh[:, 256:384], hT[:, sl], start=True, stop=True)

                r_sb = gsb.tile([128, 512], F32, tag="r_sb", name="r_sb")
                z_sb = gsb.tile([128, 512], F32, tag="z_sb", name="z_sb")
                nc.scalar.activation(out=r_sb[:], in_=r_ps[:], func=AF.Sigmoid, bias=gbias[:, 0:1])
                nc.scalar.activation(out=z_sb[:], in_=z_ps[:], func=AF.Sigmoid, bias=gbias[:, 1:2])
                t1 = gsb.tile([128, 512], F32, tag="t1", name="t1")
                nc.vector.tensor_mul(out=t1[:], in0=r_sb[:], in1=hn_ps[:])
                t2 = gsb.tile([128, 512], F32, tag="t2", name="t2")
                nc.vector.tensor_add(out=t2[:], in0=t1[:], in1=xn_ps[:])
                n_sb = gsb.tile([128, 512], F32, tag="n_sb", name="n_sb")
                nc.scalar.activation(out=n_sb[:], in_=t2[:], func=AF.Tanh, bias=gbias[:, 2:3])
                d_sb = gsb.tile([128, 512], F32, tag="d_sb", name="d_sb")
                nc.vector.tensor_sub(out=d_sb[:], in0=hT[:, sl], in1=n_sb[:])
                zd = gsb.tile([128, 512], F32, tag="zd", name="zd")
                nc.vector.tensor_mul(out=zd[:], in0=z_sb[:], in1=d_sb[:])
                nc.vector.tensor_add(out=hT[:, sl], in0=n_sb[:], in1=zd[:])

            # ---- prologue: m0 = x @ W0, both AllGathers ----
            with nc.named_scope("pro"):
                for t in range(NT):
                    emit_m_tile(t, 0, 0)
                launch_ag_a(0)
                launch_ag_b(0)

            for s in range(STEPS):
                B = s % 2
                Bn = 1 - B
                with nc.named_scope(f"step{s}"):
                    grp_ps = {}
                    gru_next = 0
                    ag_a_launched = False

                    def try_emit_gru(tch):
                        nonlocal gru_next, ag_a_launched
                        while (gru_next < NT
                               and fin_chunk[2 * gru_next] <= tch
                               and fin_chunk[2 * gru_next + 1] <= tch):
                            emit_gru_tile(gru_next)
                            if s < STEPS - 1:
                                emit_m_tile(gru_next, s + 1, Bn)
                                if gru_next == 11 and not ag_a_launched:
                                    launch_ag_a(Bn)
                                    ag_a_launched = True
                            gru_next += 1

                    for ci, (c0, ncall) in enumerate(calls):
                        half = chunks[c0][1]
                        tabl = (m_fa if half == 0 else m_fb)[B]
                        gt = gpool.tile([128, CALLCH, 128], BF16, tag="gt", name="gt")
                        nc.gpsimd.dma_gather(
                            out_ap=gt[:, :ncall, :],
                            in_ap=tabl[:, :],
                            idxs_ap=idx_t[:, c0 * 8:(c0 + ncall) * 8],
                            num_idxs=ncall * 128,
                            num_idxs_reg=ncall * 128,
                            elem_size=128,
                            single_packet=False,
                            queue_num=ci % 4,
                        )
                        it = ipool.tile([128, CALLCH, 256], BF16, tag="it", name="it")
                        nc.sync.dma_start(
                            out=it[:, :ncall, :],
                            in_=ind_in[:, c0 * 256:(c0 + ncall) * 256])
                        for j in range(ncall):
                            tch = c0 + j
                            b, p = chunks[tch]
                            g = (b // 2, p)
                            if g not in grp_ps:
                                grp_ps[g] = psA.tile([128, 512], F32, tag="aggps", name="aggps")
                            off = (b % 2) * 256
                            nc.tensor.matmul(
                                grp_ps[g][:, off:off + 256],
                                gt[:, j, :],
                                it[:, j, :],
                                start=(tch == ph_first[(b, p)]),
                                stop=(tch == ph_last[(b, p)]))
                            if tch == ph_last[(b, p)]:
                                asl = slice(b * 256, (b + 1) * 256)
                                psl = grp_ps[g][:, off:off + 256]
                                if p == first_phase[b]:
                                    nc.vector.tensor_copy(out=aggT[:, asl], in_=psl)
                                else:
                                    nc.vector.tensor_add(out=aggT[:, asl], in0=aggT[:, asl], in1=psl)
                                if b % 2 == 1 or b == NBLK - 1:
                                    grp_ps.pop(g, None)
                                if tch == fin_chunk[b]:
                                    try_emit_gru(tch)

                    try_emit_gru(TOTCH)      # safety: flush any stragglers
                    if s < STEPS - 1:
                        if not ag_a_launched:
                            launch_ag_a(Bn)
                        launch_ag_b(Bn)

        # ================= conv/MLP head =================
        with nc.named_scope("head"), tc.tile_pool(name="head_sb", bufs=1) as hsb:

            bnp = hsb.tile([128, 6], F32)
            nc.sync.dma_start(out=bnp[:], in_=bn_in[:, :])
            stA = hsb.tile([128, GPC * 6], F32)     # per-graph accum stats phase A
            stC = hsb.tile([128, GPC * 6], F32)     # per-graph accum stats phase C
            st1 = hsb.tile([128, 6], F32)
            st2 = hsb.tile([128, 6], F32)
            sqscr = hsb.tile([128, 512], F32)
            relu_t = hsb.tile([128, 512], F32)
            y2 = hsb.tile([128, GPC * 256], BF16)
            z2a = hsb.tile([128, GPC * 256], BF16)
            z2b = hsb.tile([128, GPC * 256], BF16)
            ab1 = hsb.tile([128, 6], F32)
            ab2 = hsb.tile([128, 6], F32)

            def stats_into(ps_ap, sb_ap, cols, g, path):
                # Σ comes free with the ACT PSUM->SBUF copy (accum_out at the
                # call sites); Σ² is one fused DVE square-with-accumulate
                # (PSUM x its bf16 SBUF copy — only one PSUM input allowed).
                c = g * 6 + 2 * path + 1
                nc.vector.scalar_tensor_tensor(
                    out=sqscr[:, :ps_ap.shape[-1]], in0=ps_ap, scalar=1.0,
                    in1=sb_ap, op0=ALU.bypass, op1=ALU.mult,
                    accum_out=cols[:, c:c + 1])

            def reduce_stats(cols, st):
                nc.vector.reduce_sum(
                    out=st[:, :6],
                    in_=cols[:].rearrange("p (g c) -> p c g", c=6),
                    axis=mybir.AxisListType.X)

            def bn_coeffs(st, col, g_col, b_col, nn, ab, acol):
                mean = hsb.tile([128, 1], F32, tag="bnm", name="bnm")
                nc.vector.tensor_scalar_mul(mean[:], st[:, col:col + 1], 1.0 / nn)
                var = hsb.tile([128, 1], F32, tag="bnv", name="bnv")
                nc.vector.tensor_scalar_mul(var[:], st[:, col + 1:col + 2], 1.0 / nn)
                msq = hsb.tile([128, 1], F32, tag="bnq", name="bnq")
                nc.vector.tensor_mul(out=msq[:], in0=mean[:], in1=mean[:])
                nc.vector.tensor_sub(out=var[:], in0=var[:], in1=msq[:])
                nc.vector.tensor_scalar_add(var[:], var[:], 1e-5)
                sd = hsb.tile([128, 1], F32, tag="bnsd", name="bnsd")
                nc.scalar.activation(out=sd[:], in_=var[:], func=AF.Sqrt)
                inv = hsb.tile([128, 1], F32, tag="bninv", name="bninv")
                nc.vector.reciprocal(out=inv[:], in_=sd[:])
                nc.vector.tensor_mul(out=ab[:, acol:acol + 1], in0=inv[:], in1=bnp[:, g_col:g_col + 1])
                nc.vector.tensor_mul(out=mean[:], in0=mean[:], in1=ab[:, acol:acol + 1])
                nc.vector.tensor_sub(out=ab[:, acol + 1:acol + 2], in0=bnp[:, b_col:b_col + 1], in1=mean[:])

            def bn_relu_pool3(src_ap, acol, out_ap, ab):
                # bn+relu then maxpool k=3 s=2: [*, Lp] -> [*, P1]
                nc.scalar.activation(out=relu_t[:, :Lp], in_=src_ap, func=AF.Relu,
                                     bias=ab[:, acol + 1:acol + 2], scale=ab[:, acol:acol + 1])
                a = relu_t[:, 0:2 * P1].rearrange("p (l t) -> p t l", t=2)
                bb = relu_t[:, 2:2 + 2 * P1].rearrange("p (l t) -> p t l", t=2)
                mx = hsb.tile([128, P1], F32, tag="mx", name="mx")
                nc.vector.tensor_max(out=mx[:], in0=a[:, 0, :], in1=a[:, 1, :])
                nc.vector.tensor_max(out=out_ap, in0=mx[:], in1=bb[:, 0, :])

            # ---- phase A/B: conv1+convc1, stats, bn+relu+pool ----
            with tc.tile_pool(name="pA_sb", bufs=1) as pa, \
                 tc.tile_pool(name="pA_ps", bufs=2, space="PSUM") as hps:
                c1w = pa.tile([128, 3 * 128], F32R)
                nc.gpsimd.dma_start(out=c1w[:].rearrange("a (k b) -> a k b", b=128), in_=c1w_in.rearrange("k a b -> a k b"))
                cc1w = pa.tile([128, 12 * 128], F32R)
                nc.gpsimd.dma_start(out=cc1w[:].rearrange("a (k b) -> a k b", b=128), in_=cc1w_in.rearrange("k a b -> a k b"))
                y1 = pa.tile([128, GPC * 512], BF16)
                z1a = pa.tile([128, GPC * 512], BF16)
                z1b = pa.tile([128, GPC * 512], BF16)

                for g in range(GPC):
                    gs = slice(g * 512, g * 512 + 512)
                    hg = hT[:, gs]
                    xg = xT[:, gs]
                    c1ps = hps.tile([128, 512], F32, tag="c1ps", name="c1ps")
                    for k in range(3):
                        nc.tensor.matmul(c1ps[:, :Lp], c1w[:, k * 128:(k + 1) * 128],
                                         hg[:, k:k + Lp], start=(k == 0), stop=(k == 2))
                    nc.scalar.activation(out=y1[:, g * 512:g * 512 + Lp], in_=c1ps[:, :Lp],
                                         func=AF.Copy, accum_out=stA[:, g * 6:g * 6 + 1])
                    stats_into(c1ps[:, :Lp], y1[:, g * 512:g * 512 + Lp], stA, g, 0)
                    for co in range(2):
                        ccps = hps.tile([128, 512], F32, tag="ccps", name="ccps")
                        for k in range(3):
                            nc.tensor.matmul(ccps[:, :Lp],
                                             cc1w[:, (k * 4 + co) * 128:(k * 4 + co) * 128 + 128],
                                             hg[:, k:k + Lp], start=(k == 0), stop=False)
                        for k in range(3):
                            nc.tensor.matmul(ccps[:, :Lp],
                                             cc1w[:, (k * 4 + 2 + co) * 128:(k * 4 + 2 + co) * 128 + 128],
                                             xg[:, k:k + Lp], start=False, stop=(k == 2))
                        dst = z1a if co == 0 else z1b
                        c = g * 6 + 2 * (1 + co)
                        nc.scalar.activation(out=dst[:, g * 512:g * 512 + Lp], in_=ccps[:, :Lp],
                                             func=AF.Copy, accum_out=stA[:, c:c + 1])
                        stats_into(ccps[:, :Lp], dst[:, g * 512:g * 512 + Lp], stA, g, 1 + co)

                reduce_stats(stA, st1)
                nc.sync.dma_start(out=ar1_in[:, :], in_=st1[:])
                nc.gpsimd.collective_compute("AllReduce", ALU.add, replica_groups=rg,
                                             ins=[ar1_in[:, :]], outs=[ar1_out[:, :]])
                nc.sync.dma_start(out=st1[:], in_=ar1_out[:, :])
                bn_coeffs(st1, 0, 0, 1, NN1, ab1, 0)
                bn_coeffs(st1, 2, 2, 3, NN1, ab1, 2)
                bn_coeffs(st1, 4, 4, 5, NN1, ab1, 4)

                for g in range(GPC):
                    gs = slice(g * 512, g * 512 + 512)
                    o = g * 256
                    bn_relu_pool3(y1[:, gs][:, :Lp], 0, y2[:, o:o + P1], ab1)
                    bn_relu_pool3(z1a[:, gs][:, :Lp], 2, z2a[:, o:o + P1], ab1)
                    bn_relu_pool3(z1b[:, gs][:, :Lp], 4, z2b[:, o:o + P1], ab1)

            # ---- phase C: conv2/convc2 + stats2 + bn/relu/pool + proj ----
            with tc.tile_pool(name="pC_sb", bufs=1) as pc, \
                 tc.tile_pool(name="pC_ps", bufs=2, space="PSUM") as hps:
                c2wf = pc.tile([128, 128], F32)
                nc.gpsimd.dma_start(out=c2wf[:], in_=c2w_in[:, :])
                cc2wf = pc.tile([128, 4 * 128], F32)
                nc.gpsimd.dma_start(out=cc2wf[:].rearrange("a (k b) -> a k b", b=128), in_=cc2w_in.rearrange("k a b -> a k b"))
                c2w = pc.tile([128, 128], BF16)
                nc.scalar.activation(out=c2w[:], in_=c2wf[:], func=AF.Copy)
                cc2w = pc.tile([128, 4 * 128], BF16)
                nc.scalar.activation(out=cc2w[:], in_=cc2wf[:], func=AF.Copy)
                y3 = pc.tile([128, GPC * 256], BF16)
                z3a = pc.tile([128, GPC * 256], BF16)
                z3b = pc.tile([128, GPC * 256], BF16)

                for g in range(GPC):
                    gs = slice(g * 256, g * 256 + 256)
                    c2ps = hps.tile([128, 256], F32, tag="c2ps", name="c2ps")
                    nc.tensor.matmul(c2ps[:], c2w[:], y2[:, gs], start=True, stop=True)
                    nc.scalar.activation(out=y3[:, gs][:, :P1], in_=c2ps[:, :P1],
                                         func=AF.Copy, accum_out=stC[:, g * 6:g * 6 + 1])
                    stats_into(c2ps[:, :P1], y3[:, gs][:, :P1], stC, g, 0)
                    for co in range(2):
                        ccps2 = hps.tile([128, 256], F32, tag="ccps2", name="ccps2")
                        nc.tensor.matmul(ccps2[:], cc2w[:, co * 128:co * 128 + 128],
                                         z2a[:, gs], start=True, stop=False)
                        nc.tensor.matmul(ccps2[:], cc2w[:, (2 + co) * 128:(2 + co) * 128 + 128],
                                         z2b[:, gs], start=False, stop=True)
                        dst3 = z3a if co == 0 else z3b
                        c = g * 6 + 2 * (1 + co)
                        nc.scalar.activation(out=dst3[:, gs][:, :P1], in_=ccps2[:, :P1],
                                             func=AF.Copy, accum_out=stC[:, c:c + 1])
                        stats_into(ccps2[:, :P1], dst3[:, gs][:, :P1], stC, g, 1 + co)

                reduce_stats(stC, st2)
                nc.sync.dma_start(out=ar2_in[:, :], in_=st2[:])
                nc.gpsimd.collective_compute("AllReduce", ALU.add, replica_groups=rg,
                                             ins=[ar2_in[:, :]], outs=[ar2_out[:, :]])
                nc.sync.dma_start(out=st2[:], in_=ar2_out[:, :])
                bn_coeffs(st2, 0, 0, 1, NN2, ab2, 0)
                bn_coeffs(st2, 2, 2, 3, NN2, ab2, 2)
                bn_coeffs(st2, 4, 4, 5, NN2, ab2, 4)

                mlpyf = pc.tile([128, 2], F32)
                nc.sync.dma_start(out=mlpyf[:], in_=mlpy_in[:, :])
                mlpy = pc.tile([128, 2], BF16)
                nc.scalar.activation(out=mlpy[:], in_=mlpyf[:], func=AF.Copy)
                mlpzf = pc.tile([128, 4], F32)
                nc.sync.dma_start(out=mlpzf[:], in_=mlpz_in[:, :])
                mlpz = pc.tile([128, 4], BF16)
                nc.scalar.activation(out=mlpz[:], in_=mlpzf[:], func=AF.Copy)
                mlpb = pc.tile([2, 2], F32)
                nc.sync.dma_start(out=mlpb[:], in_=mlpb_in[:, :])
                outsb = pc.tile([2, GPC], F32)
                y4 = pc.tile([128, GPC * 128], BF16)
                z4a = pc.tile([128, GPC * 128], BF16)
                z4b = pc.tile([128, GPC * 128], BF16)

                def bn_relu_pool2(src_t, gs, acol, out_ap, ab):
                    nc.scalar.activation(out=relu_t[:, :P1], in_=src_t[:, gs][:, :P1], func=AF.Relu,
                                         bias=ab[:, acol + 1:acol + 2], scale=ab[:, acol:acol + 1])
                    a = relu_t[:, 0:2 * L4].rearrange("p (l t) -> p t l", t=2)
                    nc.vector.tensor_max(out=out_ap, in0=a[:, 0, :], in1=a[:, 1, :])

                for g in range(GPC):
                    gs = slice(g * 256, g * 256 + 256)
                    bn_relu_pool2(y3, gs, 0, y4[:, g * 128:g * 128 + L4], ab2)
                    bn_relu_pool2(z3a, gs, 2, z4a[:, g * 128:g * 128 + L4], ab2)
                    bn_relu_pool2(z3b, gs, 4, z4b[:, g * 128:g * 128 + L4], ab2)

                # batched projection over 4-graph groups (512-col psum tiles)
                prod = pc.tile([128, GPC * 128], F32, tag="prod", name="prod")
                for q in range(GPC // 4):
                    qs = slice(q * 512, (q + 1) * 512)
                    yp = hps.tile([2, 512], F32, tag="yp", name="yp")
                    nc.tensor.matmul(yp[:], mlpy[:], y4[:, qs], start=True, stop=True)
                    zp = hps.tile([2, 512], F32, tag="zp", name="zp")
                    nc.tensor.matmul(zp[:], mlpz[:, 0:2], z4a[:, qs], start=True, stop=False)
                    nc.tensor.matmul(zp[:], mlpz[:, 2:4], z4b[:, qs], start=False, stop=True)
                    ypb = pc.tile([2, 512], F32, tag="ypb", name="ypb")
                    nc.vector.tensor_scalar_add(ypb[:], yp[:], mlpb[:, 0:1])
                    zpb = pc.tile([2, 512], F32, tag="zpb", name="zpb")
                    nc.vector.tensor_scalar_add(zpb[:], zp[:], mlpb[:, 1:2])
                    nc.vector.tensor_mul(out=prod[:2, qs], in0=ypb[:], in1=zpb[:])
                nc.vector.reduce_sum(
                    out=outsb[:, :GPC],
                    in_=prod[:2, :].rearrange("p (g l) -> p g l", l=128)[:, :, :L4],
                    axis=mybir.AxisListType.X)
                nc.vector.tensor_scalar_mul(outsb[:], outsb[:], 1.0 / L4)
                nc.sync.dma_start(out=out_p.rearrange("g p -> p g"), in_=outsb[:])

    nc.finalize()
    return nc


# --------------------------------------------------------------------------
# host weight packing
# --------------------------------------------------------------------------

def _make_inmaps(cfg, lay, inputs):
    N = cfg["N"]
    SH = N // NCORES
    f32 = np.float32
    x = np.asarray(inputs["x"], f32)
    wgg = np.ascontiguousarray(np.asarray(inputs["ggnn_w"], f32))
    wihT = np.ascontiguousarray(np.asarray(inputs["gru_wih"], f32).T)
    whhT = np.ascontiguousarray(np.asarray(inputs["gru_whh"], f32).T)
    bih = np.asarray(inputs["gru_bih"], f32)
    bhh = np.asarray(inputs["gru_bhh"], f32)
    gbias = np.zeros((128, 4), f32)
    gbias[:, 0] = bih[0:128] + bhh[0:128]
    gbias[:, 1] = bih[128:256] + bhh[128:256]
    gbias[:, 2] = bih[256:384]
    gbias[:, 3] = bhh[256:384]
    assert np.all(bhh[256:384] == 0), "nonzero bhh_n not supported"

    c1 = np.asarray(inputs["conv1_w"], f32)
    c1w = np.ascontiguousarray(np.transpose(c1, (2, 1, 0)))
    c2w = np.ascontiguousarray(np.asarray(inputs["conv2_w"], f32)[:, :, 0].T)
    cc1 = np.asarray(inputs["convc1_w"], f32)
    cc1w = np.zeros((12, 128, 128), f32)
    for k in range(3):
        for ci in range(2):
            for co in range(2):
                cc1w[k * 4 + ci * 2 + co] = cc1[co * 128:(co + 1) * 128,
                                                ci * 128:(ci + 1) * 128, k].T
    cc2 = np.asarray(inputs["convc2_w"], f32)[:, :, 0]
    cc2w = np.zeros((4, 128, 128), f32)
    for ci in range(2):
        for co in range(2):
            cc2w[ci * 2 + co] = cc2[co * 128:(co + 1) * 128, ci * 128:(ci + 1) * 128].T
    bnp = np.zeros((128, 6), f32)
    bnp[:, 0] = np.asarray(inputs["bn1_g"], f32)
    bnp[:, 1] = np.asarray(inputs["bn1_b"], f32)
    bn2g = np.asarray(inputs["bn2_g"], f32)
    bn2b = np.asarray(inputs["bn2_b"], f32)
    bnp[:, 2] = bn2g[:128]; bnp[:, 3] = bn2b[:128]
    bnp[:, 4] = bn2g[128:]; bnp[:, 5] = bn2b[128:]
    mlpyT = np.ascontiguousarray(np.asarray(inputs["mlpy_w"], f32).T)
    mzw = np.asarray(inputs["mlpz_w"], f32)
    mlpzT = np.zeros((128, 4), f32)
    mlpzT[:, 0:2] = mzw[:, :128].T
    mlpzT[:, 2:4] = mzw[:, 128:].T
    mlpb = np.zeros((2, 2), f32)
    mlpb[:, 0] = np.asarray(inputs["mlpy_b"], f32)
    mlpb[:, 1] = np.asarray(inputs["mlpz_b"], f32)

    common = dict(wgg=wgg, wihT=wihT, whhT=whhT, gbias=gbias, c1w=c1w, c2w=c2w,
                  cc1w=cc1w, cc2w=cc2w, bnp=bnp, mlpyT=mlpyT, mlpzT=mlpzT, mlpb=mlpb)
    in_maps = []
    for c in range(NCORES):
        xT = np.ascontiguousarray(x[c * SH:(c + 1) * SH].T)
        in_maps.append(dict(xT=xT, gidx=lay["gidx"][c], ind=lay["ind"][c], **common))
    return in_maps


def run(cfg, inputs, trace=False):
    lay = _prep_edges(cfg, inputs["edge_index"], inputs["edge_weight"])
    nc = _build(cfg, lay)
    in_maps = _make_inmaps(cfg, lay, inputs)
    res = run_bass_kernel_spmd(nc, in_maps, list(range(NCORES)), trace=trace)
    out = np.concatenate([res.results[c]["out"] for c in range(NCORES)], axis=0)
    return out.astype(np.float32), res


def kernel(**inputs) -> np.ndarray:
    out, _ = run(_full_cfg(), inputs, trace=False)
    return out


# revision 42
# speedup vs baseline: 1.2192x; 1.0050x over previous
"""DevignModel (GGNN + conv head) Trainium2 Bass kernel, 8-core SPMD.

Sharding: nodes/graphs split 8 ways (8192 nodes = 16 graphs per core).
Pipelined GGNN step: the bf16 message table is AllGathered in TWO halves
(rows = each core's first/second 4096 local nodes) so that half-0 edge
gathers only wait on AG_a. The GRU update and the NEXT step's message
matmul are interleaved into the phase-1 scatter stream tile-by-tile, so
AG_a of step s+1 launches mid-phase-1 of step s and overlaps the gather
tail. Edge aggregation: 4-queue dma_gather of message rows + PE
weighted-indicator matmul into aggT. The conv/BN/MLP head keeps BN
statistics on the ACT accumulator / GpSimd so DVE stops being the
bottleneck, with two tiny AllReduces for cross-core BatchNorm.
"""
import numpy as np
import ml_dtypes
import concourse.bass as bass
import concourse.bacc as bacc
import concourse.mybir as mybir
from concourse.tile import TileContext
from concourse.bass_utils import run_bass_kernel_spmd

F32 = mybir.dt.float32
F32R = mybir.dt.float32r
BF16 = mybir.dt.bfloat16
I16 = mybir.dt.int16
AF = mybir.ActivationFunctionType
ALU = mybir.AluOpType

NCORES = 8
CALLCH = 12          # gather-call granularity in 128-slot chunks

# --- queue-aware DMASW semaphore lane assignment -------------------------
# Tile rotates Pool-engine DMA completion sems over 8 lanes blindly; with
# multiple SWDGE queues a lane must stay bound to one queue (completions
# are only ordered within a queue). Give each queue a dedicated lane pair.
import concourse.tile_sem_assignment as _tsa

if not getattr(_tsa, "_qaware_patched", False):
    _orig_assign_tick = _tsa.TileClockTick._assign_tick

    def _assign_tick_qaware(self, inst):
        if (isinstance(inst, _tsa.DMAInst)
                and inst.engine == mybir.EngineType.Pool
                and not isinstance(inst, _tsa.bass_isa.UserSyncedRemoteDMADescs)):
            q = int(getattr(inst, "queue_num", 0) or 0)
            tog = getattr(self, "_q_tog", None)
            if tog is None:
                tog = self._q_tog = {}
            self.next_sw_dma_idx = q * 2 + tog.get(q, 0)
            tog[q] = 1 - tog.get(q, 0)
        return _orig_assign_tick(self, inst)

    _tsa.TileClockTick._assign_tick = _assign_tick_qaware
    _tsa._qaware_patched = True


def _full_cfg():
    return dict(N=65536, G=128, L=512, D=128, E=262144, STEPS=6)


# --------------------------------------------------------------------------
# host-side edge preprocessing
# --------------------------------------------------------------------------

def _prep_edges(cfg, edge_index, edge_weight):
    N, E = cfg["N"], cfg["E"]
    SH = N // NCORES
    HH = SH // 2
    NBLK = SH // 256
    src = np.asarray(edge_index[0], dtype=np.int64)
    dst = np.asarray(edge_index[1], dtype=np.int64)
    w = np.asarray(edge_weight, dtype=np.float32)

    per_core = []
    counts = np.zeros((NCORES, NBLK, 2), dtype=np.int64)
    for c in range(NCORES):
        m = (dst // SH) == c
        s, d, ww = src[m], dst[m] - c * SH, w[m]
        blk = d >> 8
        din = d & 255
        # table halves = first/second 4096 local nodes of each source core;
        # AllGather of those slices puts global node g at row
        # (g // SH) * HH + (g % HH) of its half's table (int16-safe).
        half = ((s % SH) // HH).astype(np.int64)
        row = (s // SH) * HH + (s % HH)
        order = np.lexsort((half, blk))
        per_core.append((row[order], din[order], ww[order], blk[order], half[order]))
        np.add.at(counts[c], (blk, half), 1)

    # common chunk layout: per (block, phase) the max chunk count over cores
    nch = np.ceil(counts / 128.0).astype(np.int64).max(axis=0)
    for b in range(NBLK):
        if nch[b].sum() == 0:
            nch[b, 0] = 1
    chunks = []   # phase-major (half 0 chunks first), block-ascending inside
    for p in range(2):
        for b in range(NBLK):
            for _ in range(int(nch[b, p])):
                chunks.append((b, p))
    TOTCH = len(chunks)
    TOT = TOTCH * 128

    gidx_all, ind_all = [], []
    for c in range(NCORES):
        s, din, ww, blk, par = per_core[c]
        idx_sl = np.zeros(TOT, dtype=np.int16)
        w_sl = np.zeros(TOT, dtype=np.float32)
        d_sl = np.zeros(TOT, dtype=np.int64)
        cc = np.zeros((NBLK, 2), dtype=np.int64)
        np.add.at(cc, (blk, par), 1)
        starts = {}
        off = 0
        for b in range(NBLK):
            for p in range(2):
                starts[(b, p)] = off
                off += cc[b, p]
        used = {k: 0 for k in starts}
        pos = 0
        for (b, p) in chunks:
            st = starts[(b, p)] + used[(b, p)]
            n = int(min(128, cc[b, p] - used[(b, p)]))
            if n > 0:
                sl = slice(st, st + n)
                idx_sl[pos:pos + n] = s[sl].astype(np.int16)
                w_sl[pos:pos + n] = ww[sl]
                d_sl[pos:pos + n] = din[sl]
                used[(b, p)] += n
            pos += 128
        # per-slot dst-in-block / weight, [128, TOTCH]: the on-chip indicator
        # is built per chunk as  w * (iota == dst)  with one fused DVE op.
        ind_all.append((
            np.ascontiguousarray(d_sl.reshape(TOTCH, 128).T.astype(np.float32)),
            np.ascontiguousarray(w_sl.reshape(TOTCH, 128).T.astype(np.float32)),
        ))
        gi = np.tile(idx_sl.reshape(TOT // 16, 16).T, (8, 1)).copy()
        gidx_all.append(gi)

    calls = []
    h1 = next((i for i, (b, p) in enumerate(chunks) if p == 1), TOTCH)
    for lo, hi in ((0, h1), (h1, TOTCH)):
        ch0 = lo
        while ch0 < hi:
            n = min(CALLCH, hi - ch0)
            calls.append((ch0, n))
            ch0 += n
    return dict(chunks=chunks, calls=calls, TOTCH=TOTCH,
                gidx=gidx_all, ind=ind_all, NBLK=NBLK)


# --------------------------------------------------------------------------
# kernel builder (one SPMD program)
# --------------------------------------------------------------------------

def _build(cfg, lay):
    N, G, L, D, STEPS = cfg["N"], cfg["G"], cfg["L"], cfg["D"], cfg["STEPS"]
    SH = N // NCORES
    HH = SH // 2
    GPC = G // NCORES          # graphs per core
    NBLK = lay["NBLK"]
    NT = SH // 512             # 512-node tiles per core
    TOTCH = lay["TOTCH"]
    chunks, calls = lay["chunks"], lay["calls"]
    Lp = L - 2                 # 510
    P1 = (Lp - 3) // 2 + 1     # 254
    L4 = (P1 - 2) // 2 + 1     # 127
    NN1 = float(G * Lp)
    NN2 = float(G * P1)

    nc = bacc.Bacc(None, target_bir_lowering=False, debug=False,
                   num_swdge_queues=4)

    # ---- I/O ----
    xT_in = nc.declare_dram_parameter("xT", [128, SH], F32, isOutput=False)
    gidx_in = nc.declare_dram_parameter("gidx", [128, TOTCH * 8], I16, isOutput=False)
    dst_in = nc.declare_dram_parameter("dstw", [128, TOTCH], F32, isOutput=False)
    wsl_in = nc.declare_dram_parameter("wsl", [128, TOTCH], F32, isOutput=False)
    iota_in = nc.declare_dram_parameter("iota", [128, 256], F32, isOutput=False)
    wgg_in = nc.declare_dram_parameter("wgg", [STEPS, 128, 128], F32, isOutput=False)
    wih_in = nc.declare_dram_parameter("wihT", [128, 384], F32, isOutput=False)
    whh_in = nc.declare_dram_parameter("whhT", [128, 384], F32, isOutput=False)
    gb_in = nc.declare_dram_parameter("gbias", [128, 4], F32, isOutput=False)
    c1w_in = nc.declare_dram_parameter("c1w", [3, 128, 128], F32, isOutput=False)
    c2w_in = nc.declare_dram_parameter("c2w", [128, 128], F32, isOutput=False)
    cc1w_in = nc.declare_dram_parameter("cc1w", [12, 128, 128], F32, isOutput=False)
    cc2w_in = nc.declare_dram_parameter("cc2w", [4, 128, 128], F32, isOutput=False)
    bn_in = nc.declare_dram_parameter("bnp", [128, 6], F32, isOutput=False)
    mlpy_in = nc.declare_dram_parameter("mlpyT", [128, 2], F32, isOutput=False)
    mlpz_in = nc.declare_dram_parameter("mlpzT", [128, 4], F32, isOutput=False)
    mlpb_in = nc.declare_dram_parameter("mlpb", [2, 2], F32, isOutput=False)
    out_p = nc.declare_dram_parameter("out", [GPC, 2], F32, isOutput=True)

    # ---- internal DRAM ----
    m_loc = [nc.dram_tensor(f"m_loc{i}", [SH, D], BF16) for i in range(2)]
    m_fa = [nc.dram_tensor(f"m_fa{i}", [N // 2, D], BF16, addr_space="Shared")
            for i in range(2)]
    m_fb = [nc.dram_tensor(f"m_fb{i}", [N // 2, D], BF16, addr_space="Shared")
            for i in range(2)]
    ar1_in = nc.dram_tensor("ar1_in", [128, 6], F32)
    ar1_out = nc.dram_tensor("ar1_out", [128, 6], F32)
    ar2_in = nc.dram_tensor("ar2_in", [128, 6], F32)
    ar2_out = nc.dram_tensor("ar2_out", [128, 6], F32)

    rg = [list(range(NCORES))]

    # chunk/phase metadata
    ph_first, ph_last = {}, {}
    for t, (b, p) in enumerate(chunks):
        ph_first.setdefault((b, p), t)
        ph_last[(b, p)] = t
    first_phase, last_phase, fin_chunk = {}, {}, {}
    for b in range(NBLK):
        ps = [p for p in (0, 1) if (b, p) in ph_first]
        first_phase[b] = min(ps)
        last_phase[b] = max(ps)
        fin_chunk[b] = ph_last[(b, last_phase[b])]

    with TileContext(nc) as tc:
      with tc.tile_pool(name="persist", bufs=1) as pp:
        hT = pp.tile([128, SH], F32R)
        xT = pp.tile([128, SH], F32R)
        nc.gpsimd.dma_start(out=hT[:], in_=xT_in[:, :])
        nc.gpsimd.dma_start(out=xT[:], in_=xT_in[:, :])

        # ================= GGNN =================
        with tc.tile_pool(name="ggnn_sb", bufs=1) as gsb, \
             tc.tile_pool(name="gath", bufs=4) as gpool, \
             tc.tile_pool(name="indp", bufs=8) as ipool, \
             tc.tile_pool(name="psA", bufs=2, space="PSUM") as psA, \
             tc.tile_pool(name="psB", bufs=1, space="PSUM") as psB:

            aggT = gsb.tile([128, SH], F32R)
            m_stage = gsb.tile([128, SH], BF16)
            idx_t = gsb.tile([128, TOTCH * 8], I16)
            nc.sync.dma_start(out=idx_t[:], in_=gidx_in[:, :])
            dstf = gsb.tile([128, TOTCH], F32)
            nc.sync.dma_start(out=dstf[:], in_=dst_in[:, :])
            wf = gsb.tile([128, TOTCH], F32)
            nc.sync.dma_start(out=wf[:], in_=wsl_in[:, :])
            iota = gsb.tile([128, 256], F32)
            nc.sync.dma_start(out=iota[:], in_=iota_in[:, :])
            wih = gsb.tile([128, 384], F32R)
            nc.gpsimd.dma_start(out=wih[:], in_=wih_in[:, :])
            whh = gsb.tile([128, 384], F32R)
            nc.gpsimd.dma_start(out=whh[:], in_=whh_in[:, :])
            wgg = gsb.tile([128, STEPS * 128], F32R)
            nc.gpsimd.dma_start(out=wgg[:].rearrange("k (s d) -> k s d", d=128), in_=wgg_in.rearrange("s k d -> k s d"))
            gbias = gsb.tile([128, 4], F32)
            nc.sync.dma_start(out=gbias[:], in_=gb_in[:, :])

            def emit_m_tile(t, s_next, B_next):
                # m[512t : 512t+512] = h @ W[s_next], natural [node, dout] layout
                mps = psA.tile([128, 512], F32, tag="mps", name="mps")
                for j4 in range(4):
                    n = t * 4 + j4
                    nc.tensor.matmul(
                        mps[:, j4 * 128:(j4 + 1) * 128],
                        hT[:, n * 128:(n + 1) * 128],
                        wgg[:, s_next * 128:(s_next + 1) * 128],
                        start=True, stop=True)
                nc.scalar.activation(out=m_stage[:, t * 512:(t + 1) * 512],
                                     in_=mps[:], func=AF.Copy)
                mlv = m_loc[B_next].rearrange("(n p) d -> p n d", p=128)
                msv = m_stage[:].rearrange("p (n d) -> p n d", d=128)
                nc.sync.dma_start(out=mlv[:, t * 4:(t + 1) * 4, :],
                                  in_=msv[:, t * 4:(t + 1) * 4, :])

            def launch_ag_a(B_next):
                nc.gpsimd.collective_compute(
                    "AllGather", ALU.bypass, replica_groups=rg,
                    ins=[m_loc[B_next][:HH, :]], outs=[m_fa[B_next][:, :]])

            def launch_ag_b(B_next):
                nc.gpsimd.collective_compute(
                    "AllGather", ALU.bypass, replica_groups=rg,
                    ins=[m_loc[B_next][HH:, :]], outs=[m_fb[B_next][:, :]])

            def emit_gru_tile(t):
                sl = slice(t * 512, (t + 1) * 512)
                r_ps = psB.tile([128, 512], F32, tag="rps", name="r_ps")
                z_ps = psB.tile([128, 512], F32, tag="zps", name="z_ps")
                xn_ps = psB.tile([128, 512], F32, tag="xnps", name="xn_ps")
                hn_ps = psB.tile([128, 512], F32, tag="hnps", name="hn_ps")
                nc.tensor.matmul(r_ps[:], wih[:, 0:128], aggT[:, sl], start=True, stop=False)
                nc.tensor.matmul(r_ps[:], whh[:, 0:128], hT[:, sl], start=False, stop=True)
                nc.tensor.matmul(z_ps[:], wih[:, 128:256], aggT[:, sl], start=True, stop=False)
                nc.tensor.matmul(z_ps[:], whh[:, 128:256], hT[:, sl], start=False, stop=True)
                nc.tensor.matmul(xn_ps[:], wih[:, 256:384], aggT[:, sl], start=True, stop=True)
                nc.tensor.matmul(hn_ps[:], whh[:, 256:384], hT[:, sl], start=True, stop=True)

                r_sb = gsb.tile([128, 512], F32, tag="r_sb", name="r_sb")
                z_sb = gsb.tile([128, 512], F32, tag="z_sb", name="z_sb")
                nc.scalar.activation(out=r_sb[:], in_=r_ps[:], func=AF.Sigmoid, bias=gbias[:, 0:1])
                nc.scalar.activation(out=z_sb[:], in_=z_ps[:], func=AF.Sigmoid, bias=gbias[:, 1:2])
                t1 = gsb.tile([128, 512], F32, tag="t1", name="t1")
                nc.vector.tensor_mul(out=t1[:], in0=r_sb[:], in1=hn_ps[:])
                t2 = gsb.tile([128, 512], F32, tag="t2", name="t2")
                nc.vector.tensor_add(out=t2[:], in0=t1[:], in1=xn_ps[:])
                n_sb = gsb.tile([128, 512], F32, tag="n_sb", name="n_sb")
                nc.scalar.activation(out=n_sb[:], in_=t2[:], func=AF.Tanh, bias=gbias[:, 2:3])
                d_sb = gsb.tile([128, 512], F32, tag="d_sb", name="d_sb")
                nc.vector.tensor_sub(out=d_sb[:], in0=hT[:, sl], in1=n_sb[:])
                zd = gsb.tile([128, 512], F32, tag="zd", name="zd")
                nc.vector.tensor_mul(out=zd[:], in0=z_sb[:], in1=d_sb[:])
                nc.vector.tensor_add(out=hT[:, sl], in0=n_sb[:], in1=zd[:])

            # ---- prologue: m0 = x @ W0, both AllGathers ----
            with nc.named_scope("pro"):
                for t in range(NT):
                    emit_m_tile(t, 0, 0)
                launch_ag_a(0)
                launch_ag_b(0)

            for s in range(STEPS):
                B = s % 2
                Bn = 1 - B
                with nc.named_scope(f"step{s}"):
                    grp_ps = {}
                    gru_next = 0
                    ag_a_launched = False

                    def try_emit_gru(tch):
                        nonlocal gru_next, ag_a_launched
                        while (gru_next < NT
                               and fin_chunk[2 * gru_next] <= tch
                               and fin_chunk[2 * gru_next + 1] <= tch):
                            emit_gru_tile(gru_next)
                            if s < STEPS - 1:
                                emit_m_tile(gru_next, s + 1, Bn)
                                if gru_next == 7 and not ag_a_launched:
                                    launch_ag_a(Bn)
                                    ag_a_launched = True
                            gru_next += 1

                    for ci, (c0, ncall) in enumerate(calls):
                        half = chunks[c0][1]
                        tabl = (m_fa if half == 0 else m_fb)[B]
                        gt = gpool.tile([128, CALLCH, 128], BF16, tag="gt", name="gt")
                        nc.gpsimd.dma_gather(
                            out_ap=gt[:, :ncall, :],
                            in_ap=tabl[:, :],
                            idxs_ap=idx_t[:, c0 * 8:(c0 + ncall) * 8],
                            num_idxs=ncall * 128,
                            num_idxs_reg=ncall * 128,
                            elem_size=128,
                            single_packet=False,
                            queue_num=ci % 4,
                        )
                        for j in range(ncall):
                            tch = c0 + j
                            b, p = chunks[tch]
                            g = (b // 2, p)
                            if g not in grp_ps:
                                grp_ps[g] = psA.tile([128, 512], F32, tag="aggps", name="aggps")
                            off = (b % 2) * 256
                            it = ipool.tile([128, 256], BF16, tag="it", name="it")
                            nc.vector.tensor_scalar(
                                out=it[:], in0=iota[:],
                                scalar1=dstf[:, tch:tch + 1],
                                scalar2=wf[:, tch:tch + 1],
                                op0=ALU.is_equal, op1=ALU.mult)
                            nc.tensor.matmul(
                                grp_ps[g][:, off:off + 256],
                                gt[:, j, :],
                                it[:],
                                start=(tch == ph_first[(b, p)]),
                                stop=(tch == ph_last[(b, p)]))
                            if tch == ph_last[(b, p)]:
                                asl = slice(b * 256, (b + 1) * 256)
                                psl = grp_ps[g][:, off:off + 256]
                                if p == first_phase[b]:
                                    nc.vector.tensor_copy(out=aggT[:, asl], in_=psl)
                                else:
                                    nc.vector.tensor_add(out=aggT[:, asl], in0=aggT[:, asl], in1=psl)
                                if b % 2 == 1 or b == NBLK - 1:
                                    grp_ps.pop(g, None)
                                if tch == fin_chunk[b]:
                                    try_emit_gru(tch)

                    try_emit_gru(TOTCH)      # safety: flush any stragglers
                    if s < STEPS - 1:
                        if not ag_a_launched:
                            launch_ag_a(Bn)
                        launch_ag_b(Bn)

        # ================= conv/MLP head =================
        with nc.named_scope("head"), tc.tile_pool(name="head_sb", bufs=1) as hsb:

            bnp = hsb.tile([128, 6], F32)
            nc.sync.dma_start(out=bnp[:], in_=bn_in[:, :])
            stA = hsb.tile([128, GPC * 6], F32)     # per-graph accum stats phase A
            stC = hsb.tile([128, GPC * 6], F32)     # per-graph accum stats phase C
            st1 = hsb.tile([128, 6], F32)
            st2 = hsb.tile([128, 6], F32)
            sqscr = hsb.tile([128, 512], F32)
            relu_t = hsb.tile([128, 512], F32)
            y2 = hsb.tile([128, GPC * 256], F32R)
            z2a = hsb.tile([128, GPC * 256], F32R)
            z2b = hsb.tile([128, GPC * 256], F32R)
            ab1 = hsb.tile([128, 6], F32)
            ab2 = hsb.tile([128, 6], F32)

            def stats_into(ps_ap, cols, g, path):
                # Σ comes free with the ACT PSUM->SBUF copy (accum_out at the
                # call sites); Σ² is one ACT square-with-accumulate on the
                # f32 PSUM (exact — bf16-copy-based variance costs accuracy).
                c = g * 6 + 2 * path + 1
                nc.scalar.activation(
                    out=sqscr[:, :ps_ap.shape[-1]], in_=ps_ap,
                    func=AF.Square, accum_out=cols[:, c:c + 1])

            def reduce_stats(cols, st):
                nc.vector.reduce_sum(
                    out=st[:, :6],
                    in_=cols[:].rearrange("p (g c) -> p c g", c=6),
                    axis=mybir.AxisListType.X)

            def bn_coeffs(st, col, g_col, b_col, nn, ab, acol):
                mean = hsb.tile([128, 1], F32, tag="bnm", name="bnm")
                nc.vector.tensor_scalar_mul(mean[:], st[:, col:col + 1], 1.0 / nn)
                var = hsb.tile([128, 1], F32, tag="bnv", name="bnv")
                nc.vector.tensor_scalar_mul(var[:], st[:, col + 1:col + 2], 1.0 / nn)
                msq = hsb.tile([128, 1], F32, tag="bnq", name="bnq")
                nc.vector.tensor_mul(out=msq[:], in0=mean[:], in1=mean[:])
                nc.vector.tensor_sub(out=var[:], in0=var[:], in1=msq[:])
                nc.vector.tensor_scalar_add(var[:], var[:], 1e-5)
                sd = hsb.tile([128, 1], F32, tag="bnsd", name="bnsd")
                nc.scalar.activation(out=sd[:], in_=var[:], func=AF.Sqrt)
                inv = hsb.tile([128, 1], F32, tag="bninv", name="bninv")
                nc.vector.reciprocal(out=inv[:], in_=sd[:])
                nc.vector.tensor_mul(out=ab[:, acol:acol + 1], in0=inv[:], in1=bnp[:, g_col:g_col + 1])
                nc.vector.tensor_mul(out=mean[:], in0=mean[:], in1=ab[:, acol:acol + 1])
                nc.vector.tensor_sub(out=ab[:, acol + 1:acol + 2], in0=bnp[:, b_col:b_col + 1], in1=mean[:])

            def bn_relu_pool3(src_ap, acol, out_ap, ab):
                # bn+relu then maxpool k=3 s=2: [*, Lp] -> [*, P1]
                nc.scalar.activation(out=relu_t[:, :Lp], in_=src_ap, func=AF.Relu,
                                     bias=ab[:, acol + 1:acol + 2], scale=ab[:, acol:acol + 1])
                a = relu_t[:, 0:2 * P1].rearrange("p (l t) -> p t l", t=2)
                bb = relu_t[:, 2:2 + 2 * P1].rearrange("p (l t) -> p t l", t=2)
                mx = hsb.tile([128, P1], F32, tag="mx", name="mx")
                nc.vector.tensor_max(out=mx[:], in0=a[:, 0, :], in1=a[:, 1, :])
                nc.vector.tensor_max(out=out_ap, in0=mx[:], in1=bb[:, 0, :])

            # ---- phase A/B: conv1+convc1, stats, bn+relu+pool ----
            with tc.tile_pool(name="pA_sb", bufs=1) as pa, \
                 tc.tile_pool(name="pA_ps", bufs=2, space="PSUM") as hps:
                c1w = pa.tile([128, 3 * 128], F32R)
                nc.gpsimd.dma_start(out=c1w[:].rearrange("a (k b) -> a k b", b=128), in_=c1w_in.rearrange("k a b -> a k b"))
                cc1w = pa.tile([128, 12 * 128], F32R)
                nc.gpsimd.dma_start(out=cc1w[:].rearrange("a (k b) -> a k b", b=128), in_=cc1w_in.rearrange("k a b -> a k b"))
                y1 = pa.tile([128, GPC * 512], BF16)
                z1a = pa.tile([128, GPC * 512], BF16)
                z1b = pa.tile([128, GPC * 512], BF16)

                for g in range(GPC):
                    gs = slice(g * 512, g * 512 + 512)
                    hg = hT[:, gs]
                    xg = xT[:, gs]
                    c1ps = hps.tile([128, 512], F32, tag="c1ps", name="c1ps")
                    for k in range(3):
                        nc.tensor.matmul(c1ps[:, :Lp], c1w[:, k * 128:(k + 1) * 128],
                                         hg[:, k:k + Lp], start=(k == 0), stop=(k == 2))
                    nc.scalar.activation(out=y1[:, g * 512:g * 512 + Lp], in_=c1ps[:, :Lp],
                                         func=AF.Copy, accum_out=stA[:, g * 6:g * 6 + 1])
                    stats_into(c1ps[:, :Lp], stA, g, 0)
                    for co in range(2):
                        ccps = hps.tile([128, 512], F32, tag="ccps", name="ccps")
                        for k in range(3):
                            nc.tensor.matmul(ccps[:, :Lp],
                                             cc1w[:, (k * 4 + co) * 128:(k * 4 + co) * 128 + 128],
                                             hg[:, k:k + Lp], start=(k == 0), stop=False)
                        for k in range(3):
                            nc.tensor.matmul(ccps[:, :Lp],
                                             cc1w[:, (k * 4 + 2 + co) * 128:(k * 4 + 2 + co) * 128 + 128],
                                             xg[:, k:k + Lp], start=False, stop=(k == 2))
                        dst = z1a if co == 0 else z1b
                        c = g * 6 + 2 * (1 + co)
                        nc.scalar.activation(out=dst[:, g * 512:g * 512 + Lp], in_=ccps[:, :Lp],
                                             func=AF.Copy, accum_out=stA[:, c:c + 1])
                        stats_into(ccps[:, :Lp], stA, g, 1 + co)

                reduce_stats(stA, st1)
                nc.sync.dma_start(out=ar1_in[:, :], in_=st1[:])
                nc.gpsimd.collective_compute("AllReduce", ALU.add, replica_groups=rg,
                                             ins=[ar1_in[:, :]], outs=[ar1_out[:, :]])
                nc.sync.dma_start(out=st1[:], in_=ar1_out[:, :])
                bn_coeffs(st1, 0, 0, 1, NN1, ab1, 0)
                bn_coeffs(st1, 2, 2, 3, NN1, ab1, 2)
                bn_coeffs(st1, 4, 4, 5, NN1, ab1, 4)

                for g in range(GPC):
                    gs = slice(g * 512, g * 512 + 512)
                    o = g * 256
                    bn_relu_pool3(y1[:, gs][:, :Lp], 0, y2[:, o:o + P1], ab1)
                    bn_relu_pool3(z1a[:, gs][:, :Lp], 2, z2a[:, o:o + P1], ab1)
                    bn_relu_pool3(z1b[:, gs][:, :Lp], 4, z2b[:, o:o + P1], ab1)

            # ---- phase C: conv2/convc2 + stats2 + bn/relu/pool + proj ----
            with tc.tile_pool(name="pC_sb", bufs=1) as pc, \
                 tc.tile_pool(name="pC_ps", bufs=2, space="PSUM") as hps:
                c2w = pc.tile([128, 128], F32R)
                nc.gpsimd.dma_start(out=c2w[:], in_=c2w_in[:, :])
                cc2w = pc.tile([128, 4 * 128], F32R)
                nc.gpsimd.dma_start(out=cc2w[:].rearrange("a (k b) -> a k b", b=128), in_=cc2w_in.rearrange("k a b -> a k b"))
                y3 = pc.tile([128, GPC * 256], F32)
                z3a = pc.tile([128, GPC * 256], F32)
                z3b = pc.tile([128, GPC * 256], F32)

                for g in range(GPC):
                    gs = slice(g * 256, g * 256 + 256)
                    c2ps = hps.tile([128, 256], F32, tag="c2ps", name="c2ps")
                    nc.tensor.matmul(c2ps[:], c2w[:], y2[:, gs], start=True, stop=True)
                    nc.scalar.activation(out=y3[:, gs][:, :P1], in_=c2ps[:, :P1],
                                         func=AF.Copy, accum_out=stC[:, g * 6:g * 6 + 1])
                    stats_into(c2ps[:, :P1], stC, g, 0)
                    for co in range(2):
                        ccps2 = hps.tile([128, 256], F32, tag="ccps2", name="ccps2")
                        nc.tensor.matmul(ccps2[:], cc2w[:, co * 128:co * 128 + 128],
                                         z2a[:, gs], start=True, stop=False)
                        nc.tensor.matmul(ccps2[:], cc2w[:, (2 + co) * 128:(2 + co) * 128 + 128],
                                         z2b[:, gs], start=False, stop=True)
                        dst3 = z3a if co == 0 else z3b
                        c = g * 6 + 2 * (1 + co)
                        nc.scalar.activation(out=dst3[:, gs][:, :P1], in_=ccps2[:, :P1],
                                             func=AF.Copy, accum_out=stC[:, c:c + 1])
                        stats_into(ccps2[:, :P1], stC, g, 1 + co)

                reduce_stats(stC, st2)
                nc.sync.dma_start(out=ar2_in[:, :], in_=st2[:])
                nc.gpsimd.collective_compute("AllReduce", ALU.add, replica_groups=rg,
                                             ins=[ar2_in[:, :]], outs=[ar2_out[:, :]])
                nc.sync.dma_start(out=st2[:], in_=ar2_out[:, :])
                bn_coeffs(st2, 0, 0, 1, NN2, ab2, 0)
                bn_coeffs(st2, 2, 2, 3, NN2, ab2, 2)
                bn_coeffs(st2, 4, 4, 5, NN2, ab2, 4)

                mlpy = pc.tile([128, 2], F32R)
                nc.gpsimd.dma_start(out=mlpy[:], in_=mlpy_in[:, :])
                mlpz = pc.tile([128, 4], F32R)
                nc.gpsimd.dma_start(out=mlpz[:], in_=mlpz_in[:, :])
                mlpb = pc.tile([2, 2], F32)
                nc.sync.dma_start(out=mlpb[:], in_=mlpb_in[:, :])
                outsb = pc.tile([2, GPC], F32)
                y4 = pc.tile([128, GPC * 128], F32R)
                z4a = pc.tile([128, GPC * 128], F32R)
                z4b = pc.tile([128, GPC * 128], F32R)

                def bn_relu_pool2(src_t, gs, acol, out_ap, ab):
                    nc.scalar.activation(out=relu_t[:, :P1], in_=src_t[:, gs][:, :P1], func=AF.Relu,
                                         bias=ab[:, acol + 1:acol + 2], scale=ab[:, acol:acol + 1])
                    a = relu_t[:, 0:2 * L4].rearrange("p (l t) -> p t l", t=2)
                    nc.vector.tensor_max(out=out_ap, in0=a[:, 0, :], in1=a[:, 1, :])

                for g in range(GPC):
                    gs = slice(g * 256, g * 256 + 256)
                    bn_relu_pool2(y3, gs, 0, y4[:, g * 128:g * 128 + L4], ab2)
                    bn_relu_pool2(z3a, gs, 2, z4a[:, g * 128:g * 128 + L4], ab2)
                    bn_relu_pool2(z3b, gs, 4, z4b[:, g * 128:g * 128 + L4], ab2)

                # batched projection over 4-graph groups (512-col psum tiles)
                prod = pc.tile([128, GPC * 128], F32, tag="prod", name="prod")
                for q in range(GPC // 4):
                    qs = slice(q * 512, (q + 1) * 512)
                    yp = hps.tile([2, 512], F32, tag="yp", name="yp")
                    nc.tensor.matmul(yp[:], mlpy[:], y4[:, qs], start=True, stop=True)
                    zp = hps.tile([2, 512], F32, tag="zp", name="zp")
                    nc.tensor.matmul(zp[:], mlpz[:, 0:2], z4a[:, qs], start=True, stop=False)
                    nc.tensor.matmul(zp[:], mlpz[:, 2:4], z4b[:, qs], start=False, stop=True)
                    ypb = pc.tile([2, 512], F32, tag="ypb", name="ypb")
                    nc.vector.tensor_scalar_add(ypb[:], yp[:], mlpb[:, 0:1])
                    zpb = pc.tile([2, 512], F32, tag="zpb", name="zpb")
                    nc.vector.tensor_scalar_add(zpb[:], zp[:], mlpb[:, 1:2])
                    nc.vector.tensor_mul(out=prod[:2, qs], in0=ypb[:], in1=zpb[:])
                nc.vector.reduce_sum(
                    out=outsb[:, :GPC],
                    in_=prod[:2, :].rearrange("p (g l) -> p g l", l=128)[:, :, :L4],
                    axis=mybir.AxisListType.X)
                nc.vector.tensor_scalar_mul(outsb[:], outsb[:], 1.0 / L4)
                nc.sync.dma_start(out=out_p.rearrange("g p -> p g"), in_=outsb[:])

    nc.finalize()
    return nc


# --------------------------------------------------------------------------
# host weight packing
# --------------------------------------------------------------------------

def _make_inmaps(cfg, lay, inputs):
    N = cfg["N"]
    SH = N // NCORES
    f32 = np.float32
    x = np.asarray(inputs["x"], f32)
    wgg = np.ascontiguousarray(np.asarray(inputs["ggnn_w"], f32))
    wihT = np.ascontiguousarray(np.asarray(inputs["gru_wih"], f32).T)
    whhT = np.ascontiguousarray(np.asarray(inputs["gru_whh"], f32).T)
    bih = np.asarray(inputs["gru_bih"], f32)
    bhh = np.asarray(inputs["gru_bhh"], f32)
    gbias = np.zeros((128, 4), f32)
    gbias[:, 0] = bih[0:128] + bhh[0:128]
    gbias[:, 1] = bih[128:256] + bhh[128:256]
    gbias[:, 2] = bih[256:384]
    gbias[:, 3] = bhh[256:384]
    assert np.all(bhh[256:384] == 0), "nonzero bhh_n not supported"

    c1 = np.asarray(inputs["conv1_w"], f32)
    c1w = np.ascontiguousarray(np.transpose(c1, (2, 1, 0)))
    c2w = np.ascontiguousarray(np.asarray(inputs["conv2_w"], f32)[:, :, 0].T)
    cc1 = np.asarray(inputs["convc1_w"], f32)
    cc1w = np.zeros((12, 128, 128), f32)
    for k in range(3):
        for ci in range(2):
            for co in range(2):
                cc1w[k * 4 + ci * 2 + co] = cc1[co * 128:(co + 1) * 128,
                                                ci * 128:(ci + 1) * 128, k].T
    cc2 = np.asarray(inputs["convc2_w"], f32)[:, :, 0]
    cc2w = np.zeros((4, 128, 128), f32)
    for ci in range(2):
        for co in range(2):
            cc2w[ci * 2 + co] = cc2[co * 128:(co + 1) * 128, ci * 128:(ci + 1) * 128].T
    bnp = np.zeros((128, 6), f32)
    bnp[:, 0] = np.asarray(inputs["bn1_g"], f32)
    bnp[:, 1] = np.asarray(inputs["bn1_b"], f32)
    bn2g = np.asarray(inputs["bn2_g"], f32)
    bn2b = np.asarray(inputs["bn2_b"], f32)
    bnp[:, 2] = bn2g[:128]; bnp[:, 3] = bn2b[:128]
    bnp[:, 4] = bn2g[128:]; bnp[:, 5] = bn2b[128:]
    mlpyT = np.ascontiguousarray(np.asarray(inputs["mlpy_w"], f32).T)
    mzw = np.asarray(inputs["mlpz_w"], f32)
    mlpzT = np.zeros((128, 4), f32)
    mlpzT[:, 0:2] = mzw[:, :128].T
    mlpzT[:, 2:4] = mzw[:, 128:].T
    mlpb = np.zeros((2, 2), f32)
    mlpb[:, 0] = np.asarray(inputs["mlpy_b"], f32)
    mlpb[:, 1] = np.asarray(inputs["mlpz_b"], f32)

    iota = np.broadcast_to(np.arange(256, dtype=f32), (128, 256)).copy()
    common = dict(wgg=wgg, wihT=wihT, whhT=whhT, gbias=gbias, c1w=c1w, c2w=c2w,
                  cc1w=cc1w, cc2w=cc2w, bnp=bnp, mlpyT=mlpyT, mlpzT=mlpzT,
                  mlpb=mlpb, iota=iota)
    in_maps = []
    for c in range(NCORES):
        xT = np.ascontiguousarray(x[c * SH:(c + 1) * SH].T)
        dstw, wsl = lay["ind"][c]
        in_maps.append(dict(xT=xT, gidx=lay["gidx"][c], dstw=dstw, wsl=wsl,
                            **common))
    return in_maps


def run(cfg, inputs, trace=False):
    lay = _prep_edges(cfg, inputs["edge_index"], inputs["edge_weight"])
    nc = _build(cfg, lay)
    in_maps = _make_inmaps(cfg, lay, inputs)
    res = run_bass_kernel_spmd(nc, in_maps, list(range(NCORES)), trace=trace)
    out = np.concatenate([res.results[c]["out"] for c in range(NCORES)], axis=0)
    return out.astype(np.float32), res


def kernel(**inputs) -> np.ndarray:
    out, _ = run(_full_cfg(), inputs, trace=False)
    return out


# revision 50
# speedup vs baseline: 1.7780x; 1.4584x over previous
"""DevignModel (GGNN + conv head) Trainium2 Bass kernel, 8-core SPMD.

Sharding: nodes/graphs split 8 ways (8192 nodes = 16 graphs per core).
Pipelined GGNN step: the bf16 message table is AllGathered in TWO halves
(rows = each core's first/second 4096 local nodes) so that half-0 edge
gathers only wait on AG_a. The GRU update and the NEXT step's message
matmul are interleaved into the phase-1 scatter stream tile-by-tile, so
AG_a of step s+1 launches mid-phase-1 of step s and overlaps the gather
tail. Edge aggregation: 4-queue dma_gather of message rows + PE
weighted-indicator matmul into aggT. The conv/BN/MLP head keeps BN
statistics on the ACT accumulator / GpSimd so DVE stops being the
bottleneck, with two tiny AllReduces for cross-core BatchNorm.
"""
import numpy as np
import ml_dtypes
import concourse.bass as bass
import concourse.bacc as bacc
import concourse.mybir as mybir
from concourse.tile import TileContext
from concourse.bass_utils import run_bass_kernel_spmd

F32 = mybir.dt.float32
F32R = mybir.dt.float32r
BF16 = mybir.dt.bfloat16
I16 = mybir.dt.int16
AF = mybir.ActivationFunctionType
ALU = mybir.AluOpType

NCORES = 8
CALLCH = 12          # gather-call granularity in 128-slot chunks

# --- queue-aware DMASW semaphore lane assignment -------------------------
# Tile rotates Pool-engine DMA completion sems over 8 lanes blindly; with
# multiple SWDGE queues a lane must stay bound to one queue (completions
# are only ordered within a queue). Give each queue a dedicated lane pair.
import concourse.tile_sem_assignment as _tsa

if not getattr(_tsa, "_qaware_patched", False):
    _orig_assign_tick = _tsa.TileClockTick._assign_tick

    def _assign_tick_qaware(self, inst):
        if (isinstance(inst, _tsa.DMAInst)
                and inst.engine == mybir.EngineType.Pool
                and not isinstance(inst, _tsa.bass_isa.UserSyncedRemoteDMADescs)):
            q = int(getattr(inst, "queue_num", 0) or 0)
            tog = getattr(self, "_q_tog", None)
            if tog is None:
                tog = self._q_tog = {}
            self.next_sw_dma_idx = q * 2 + tog.get(q, 0)
            tog[q] = 1 - tog.get(q, 0)
        return _orig_assign_tick(self, inst)

    _tsa.TileClockTick._assign_tick = _assign_tick_qaware
    _tsa._qaware_patched = True


def _full_cfg():
    return dict(N=65536, G=128, L=512, D=128, E=262144, STEPS=6)


# --------------------------------------------------------------------------
# host-side edge preprocessing
# --------------------------------------------------------------------------

def _prep_edges(cfg, edge_index, edge_weight):
    N, E = cfg["N"], cfg["E"]
    SH = N // NCORES
    HH = SH // 2
    NBLK = SH // 256
    src = np.asarray(edge_index[0], dtype=np.int64)
    dst = np.asarray(edge_index[1], dtype=np.int64)
    w = np.asarray(edge_weight, dtype=np.float32)

    per_core = []
    counts = np.zeros((NCORES, NBLK, 2), dtype=np.int64)
    for c in range(NCORES):
        m = (dst // SH) == c
        s, d, ww = src[m], dst[m] - c * SH, w[m]
        blk = d >> 8
        din = d & 255
        # table halves = first/second 4096 local nodes of each source core;
        # AllGather of those slices puts global node g at row
        # (g // SH) * HH + (g % HH) of its half's table (int16-safe).
        half = ((s % SH) // HH).astype(np.int64)
        row = (s // SH) * HH + (s % HH)
        order = np.lexsort((half, blk))
        per_core.append((row[order], din[order], ww[order], blk[order], half[order]))
        np.add.at(counts[c], (blk, half), 1)

    # common chunk layout: per (block, phase) the max chunk count over cores
    nch = np.ceil(counts / 128.0).astype(np.int64).max(axis=0)
    for b in range(NBLK):
        if nch[b].sum() == 0:
            nch[b, 0] = 1
    chunks = []   # phase-major (half 0 chunks first), block-ascending inside
    for p in range(2):
        for b in range(NBLK):
            for _ in range(int(nch[b, p])):
                chunks.append((b, p))
    TOTCH = len(chunks)
    TOT = TOTCH * 128

    gidx_all, ind_all = [], []
    for c in range(NCORES):
        s, din, ww, blk, par = per_core[c]
        idx_sl = np.zeros(TOT, dtype=np.int16)
        w_sl = np.zeros(TOT, dtype=np.float32)
        d_sl = np.zeros(TOT, dtype=np.int64)
        cc = np.zeros((NBLK, 2), dtype=np.int64)
        np.add.at(cc, (blk, par), 1)
        starts = {}
        off = 0
        for b in range(NBLK):
            for p in range(2):
                starts[(b, p)] = off
                off += cc[b, p]
        used = {k: 0 for k in starts}
        pos = 0
        for (b, p) in chunks:
            st = starts[(b, p)] + used[(b, p)]
            n = int(min(128, cc[b, p] - used[(b, p)]))
            if n > 0:
                sl = slice(st, st + n)
                idx_sl[pos:pos + n] = s[sl].astype(np.int16)
                w_sl[pos:pos + n] = ww[sl]
                d_sl[pos:pos + n] = din[sl]
                used[(b, p)] += n
            pos += 128
        ind = np.zeros((TOT, 256), dtype=np.float32)
        ind[np.arange(TOT), d_sl] = w_sl
        # [(c e), d] -> [e, (c d)] so each gather-call's slice is contiguous per partition
        indT = np.ascontiguousarray(
            ind.reshape(-1, 128, 256).transpose(1, 0, 2).reshape(128, -1))
        ind_all.append(indT.astype(ml_dtypes.bfloat16))
        gi = np.tile(idx_sl.reshape(TOT // 16, 16).T, (8, 1)).copy()
        gidx_all.append(gi)

    # one gather call per (block, phase) group, with num_idxs trimmed to the
    # worst-core true edge count (pad slots beyond it are never gathered; the
    # zero indicator columns neutralize the stale gt rows in the matmul).
    cmax = counts.max(axis=0)     # [NBLK, 2]
    calls = []
    t = 0
    for p in range(2):
        for b in range(NBLK):
            nchk = int(nch[b, p])
            if nchk == 0:
                continue
            nidx = max(16, -(-int(cmax[b, p]) // 16) * 16)
            calls.append((t, nchk, nidx))
            t += nchk
    assert t == TOTCH
    return dict(chunks=chunks, calls=calls, TOTCH=TOTCH,
                gidx=gidx_all, ind=ind_all, NBLK=NBLK)


# --------------------------------------------------------------------------
# kernel builder (one SPMD program)
# --------------------------------------------------------------------------

def _build(cfg, lay):
    N, G, L, D, STEPS = cfg["N"], cfg["G"], cfg["L"], cfg["D"], cfg["STEPS"]
    SH = N // NCORES
    HH = SH // 2
    GPC = G // NCORES          # graphs per core
    NBLK = lay["NBLK"]
    NT = SH // 512             # 512-node tiles per core
    TOTCH = lay["TOTCH"]
    chunks, calls = lay["chunks"], lay["calls"]
    Lp = L - 2                 # 510
    P1 = (Lp - 3) // 2 + 1     # 254
    L4 = (P1 - 2) // 2 + 1     # 127
    NN1 = float(G * Lp)
    NN2 = float(G * P1)

    nc = bacc.Bacc(None, target_bir_lowering=False, debug=False,
                   num_swdge_queues=4)

    # ---- I/O ----
    xT_in = nc.declare_dram_parameter("xT", [128, SH], F32, isOutput=False)
    gidx_in = nc.declare_dram_parameter("gidx", [128, TOTCH * 8], I16, isOutput=False)
    ind_in = nc.declare_dram_parameter("ind", [128, TOTCH * 256], BF16, isOutput=False)
    wgg_in = nc.declare_dram_parameter("wgg", [STEPS, 128, 128], F32, isOutput=False)
    wih_in = nc.declare_dram_parameter("wihT", [128, 384], F32, isOutput=False)
    whh_in = nc.declare_dram_parameter("whhT", [128, 384], F32, isOutput=False)
    gb_in = nc.declare_dram_parameter("gbias", [128, 4], F32, isOutput=False)
    c1w_in = nc.declare_dram_parameter("c1w", [3, 128, 128], F32, isOutput=False)
    c2w_in = nc.declare_dram_parameter("c2w", [128, 128], F32, isOutput=False)
    cc1w_in = nc.declare_dram_parameter("cc1w", [12, 128, 128], F32, isOutput=False)
    cc2w_in = nc.declare_dram_parameter("cc2w", [4, 128, 128], F32, isOutput=False)
    bn_in = nc.declare_dram_parameter("bnp", [128, 6], F32, isOutput=False)
    mlpy_in = nc.declare_dram_parameter("mlpyT", [128, 2], F32, isOutput=False)
    mlpz_in = nc.declare_dram_parameter("mlpzT", [128, 4], F32, isOutput=False)
    mlpb_in = nc.declare_dram_parameter("mlpb", [2, 2], F32, isOutput=False)
    out_p = nc.declare_dram_parameter("out", [GPC, 2], F32, isOutput=True)

    # ---- internal DRAM ----
    m_loc = [nc.dram_tensor(f"m_loc{i}", [SH, D], BF16) for i in range(2)]
    m_fa = [nc.dram_tensor(f"m_fa{i}", [N // 2, D], BF16, addr_space="Shared")
            for i in range(2)]
    m_fb = [nc.dram_tensor(f"m_fb{i}", [N // 2, D], BF16, addr_space="Shared")
            for i in range(2)]
    ar1_in = nc.dram_tensor("ar1_in", [128, 6], F32)
    ar1_out = nc.dram_tensor("ar1_out", [128, 6], F32)
    ar2_in = nc.dram_tensor("ar2_in", [128, 6], F32)
    ar2_out = nc.dram_tensor("ar2_out", [128, 6], F32)

    rg = [list(range(NCORES))]

    # chunk/phase metadata
    ph_first, ph_last = {}, {}
    for t, (b, p) in enumerate(chunks):
        ph_first.setdefault((b, p), t)
        ph_last[(b, p)] = t
    first_phase, last_phase, fin_chunk = {}, {}, {}
    for b in range(NBLK):
        ps = [p for p in (0, 1) if (b, p) in ph_first]
        first_phase[b] = min(ps)
        last_phase[b] = max(ps)
        fin_chunk[b] = ph_last[(b, last_phase[b])]

    with TileContext(nc) as tc:
      with tc.tile_pool(name="persist", bufs=1) as pp:
        hT = pp.tile([128, SH], F32R)
        xT = pp.tile([128, SH], F32R)
        nc.gpsimd.dma_start(out=hT[:], in_=xT_in[:, :])
        nc.gpsimd.dma_start(out=xT[:], in_=xT_in[:, :])

        # ================= GGNN =================
        GMAX = max(ncall for _, ncall, _ in calls)
        with tc.tile_pool(name="ggnn_sb", bufs=1) as gsb, \
             tc.tile_pool(name="gath", bufs=6) as gpool, \
             tc.tile_pool(name="indp", bufs=6) as ipool, \
             tc.tile_pool(name="psA", bufs=2, space="PSUM") as psA, \
             tc.tile_pool(name="psB", bufs=1, space="PSUM") as psB:

            aggT = gsb.tile([128, SH], F32R)
            m_stage = gsb.tile([128, SH], BF16)
            idx_t = gsb.tile([128, TOTCH * 8], I16)
            nc.sync.dma_start(out=idx_t[:], in_=gidx_in[:, :])
            wih = gsb.tile([128, 384], F32R)
            nc.gpsimd.dma_start(out=wih[:], in_=wih_in[:, :])
            whh = gsb.tile([128, 384], F32R)
            nc.gpsimd.dma_start(out=whh[:], in_=whh_in[:, :])
            wgg = gsb.tile([128, STEPS * 128], F32R)
            nc.gpsimd.dma_start(out=wgg[:].rearrange("k (s d) -> k s d", d=128), in_=wgg_in.rearrange("s k d -> k s d"))
            gbias = gsb.tile([128, 4], F32)
            nc.sync.dma_start(out=gbias[:], in_=gb_in[:, :])

            def emit_m_tile(t, s_next, B_next):
                # m[512t : 512t+512] = h @ W[s_next], natural [node, dout] layout
                mps = psA.tile([128, 512], F32, tag="mps", name="mps")
                for j4 in range(4):
                    n = t * 4 + j4
                    nc.tensor.matmul(
                        mps[:, j4 * 128:(j4 + 1) * 128],
                        hT[:, n * 128:(n + 1) * 128],
                        wgg[:, s_next * 128:(s_next + 1) * 128],
                        start=True, stop=True)
                nc.scalar.activation(out=m_stage[:, t * 512:(t + 1) * 512],
                                     in_=mps[:], func=AF.Copy)
                mlv = m_loc[B_next].rearrange("(n p) d -> p n d", p=128)
                msv = m_stage[:].rearrange("p (n d) -> p n d", d=128)
                nc.sync.dma_start(out=mlv[:, t * 4:(t + 1) * 4, :],
                                  in_=msv[:, t * 4:(t + 1) * 4, :])

            def launch_ag_a(B_next):
                nc.gpsimd.collective_compute(
                    "AllGather", ALU.bypass, replica_groups=rg,
                    ins=[m_loc[B_next][:HH, :]], outs=[m_fa[B_next][:, :]])

            def launch_ag_b(B_next):
                nc.gpsimd.collective_compute(
                    "AllGather", ALU.bypass, replica_groups=rg,
                    ins=[m_loc[B_next][HH:, :]], outs=[m_fb[B_next][:, :]])

            def emit_gru_tile(t):
                sl = slice(t * 512, (t + 1) * 512)
                r_ps = psB.tile([128, 512], F32, tag="rps", name="r_ps")
                z_ps = psB.tile([128, 512], F32, tag="zps", name="z_ps")
                xn_ps = psB.tile([128, 512], F32, tag="xnps", name="xn_ps")
                hn_ps = psB.tile([128, 512], F32, tag="hnps", name="hn_ps")
                nc.tensor.matmul(r_ps[:], wih[:, 0:128], aggT[:, sl], start=True, stop=False)
                nc.tensor.matmul(r_ps[:], whh[:, 0:128], hT[:, sl], start=False, stop=True)
                nc.tensor.matmul(z_ps[:], wih[:, 128:256], aggT[:, sl], start=True, stop=False)
                nc.tensor.matmul(z_ps[:], whh[:, 128:256], hT[:, sl], start=False, stop=True)
                nc.tensor.matmul(xn_ps[:], wih[:, 256:384], aggT[:, sl], start=True, stop=True)
                nc.tensor.matmul(hn_ps[:], whh[:, 256:384], hT[:, sl], start=True, stop=True)

                r_sb = gsb.tile([128, 512], F32, tag="r_sb", name="r_sb")
                z_sb = gsb.tile([128, 512], F32, tag="z_sb", name="z_sb")
                nc.scalar.activation(out=r_sb[:], in_=r_ps[:], func=AF.Sigmoid, bias=gbias[:, 0:1])
                nc.scalar.activation(out=z_sb[:], in_=z_ps[:], func=AF.Sigmoid, bias=gbias[:, 1:2])
                t1 = gsb.tile([128, 512], F32, tag="t1", name="t1")
                nc.vector.tensor_mul(out=t1[:], in0=r_sb[:], in1=hn_ps[:])
                t2 = gsb.tile([128, 512], F32, tag="t2", name="t2")
                nc.vector.tensor_add(out=t2[:], in0=t1[:], in1=xn_ps[:])
                n_sb = gsb.tile([128, 512], F32, tag="n_sb", name="n_sb")
                nc.scalar.activation(out=n_sb[:], in_=t2[:], func=AF.Tanh, bias=gbias[:, 2:3])
                d_sb = gsb.tile([128, 512], F32, tag="d_sb", name="d_sb")
                nc.vector.tensor_sub(out=d_sb[:], in0=hT[:, sl], in1=n_sb[:])
                zd = gsb.tile([128, 512], F32, tag="zd", name="zd")
                nc.vector.tensor_mul(out=zd[:], in0=z_sb[:], in1=d_sb[:])
                nc.vector.tensor_add(out=hT[:, sl], in0=n_sb[:], in1=zd[:])

            # ---- prologue: m0 = x @ W0, both AllGathers ----
            with nc.named_scope("pro"):
                for t in range(NT):
                    emit_m_tile(t, 0, 0)
                launch_ag_a(0)
                launch_ag_b(0)

            for s in range(STEPS):
                B = s % 2
                Bn = 1 - B
                with nc.named_scope(f"step{s}"):
                    grp_ps = {}
                    gru_next = 0
                    ag_a_launched = False

                    def try_emit_gru(tch):
                        nonlocal gru_next, ag_a_launched
                        while (gru_next < NT
                               and fin_chunk[2 * gru_next] <= tch
                               and fin_chunk[2 * gru_next + 1] <= tch):
                            emit_gru_tile(gru_next)
                            if s < STEPS - 1:
                                emit_m_tile(gru_next, s + 1, Bn)
                                if gru_next == 7 and not ag_a_launched:
                                    launch_ag_a(Bn)
                                    ag_a_launched = True
                            gru_next += 1

                    for ci, (c0, ncall, nidx) in enumerate(calls):
                        half = chunks[c0][1]
                        tabl = (m_fa if half == 0 else m_fb)[B]
                        it = ipool.tile([128, GMAX, 256], BF16, tag="it", name="it")
                        nc.sync.dma_start(
                            out=it[:, :ncall, :],
                            in_=ind_in[:, c0 * 256:(c0 + ncall) * 256])
                        gt = gpool.tile([128, GMAX, 128], BF16, tag="gt", name="gt")
                        wch = (nidx + 127) // 128
                        nc.gpsimd.dma_gather(
                            out_ap=gt[:, :wch, :],
                            in_ap=tabl[:, :],
                            idxs_ap=idx_t[:, c0 * 8:c0 * 8 + (nidx + 15) // 16],
                            num_idxs=nidx,
                            num_idxs_reg=nidx,
                            elem_size=128,
                            single_packet=False,
                            queue_num=ci % 4,
                        )
                        for j in range(ncall):
                            tch = c0 + j
                            b, p = chunks[tch]
                            g = (b // 2, p)
                            if g not in grp_ps:
                                grp_ps[g] = psA.tile([128, 512], F32, tag="aggps", name="aggps")
                            off = (b % 2) * 256
                            nc.tensor.matmul(
                                grp_ps[g][:, off:off + 256],
                                gt[:, j, :],
                                it[:, j, :],
                                start=(tch == ph_first[(b, p)]),
                                stop=(tch == ph_last[(b, p)]))
                            if tch == ph_last[(b, p)]:
                                asl = slice(b * 256, (b + 1) * 256)
                                psl = grp_ps[g][:, off:off + 256]
                                if p == first_phase[b]:
                                    nc.vector.tensor_copy(out=aggT[:, asl], in_=psl)
                                else:
                                    nc.vector.tensor_add(out=aggT[:, asl], in0=aggT[:, asl], in1=psl)
                                if b % 2 == 1 or b == NBLK - 1:
                                    grp_ps.pop(g, None)
                                if tch == fin_chunk[b]:
                                    try_emit_gru(tch)

                    try_emit_gru(TOTCH)      # safety: flush any stragglers
                    if s < STEPS - 1:
                        if not ag_a_launched:
                            launch_ag_a(Bn)
                        launch_ag_b(Bn)

        # ================= conv/MLP head =================
        with nc.named_scope("head"), tc.tile_pool(name="head_sb", bufs=1) as hsb:

            bnp = hsb.tile([128, 6], F32)
            nc.sync.dma_start(out=bnp[:], in_=bn_in[:, :])
            stA = hsb.tile([128, GPC * 6], F32)     # per-graph accum stats phase A
            stC = hsb.tile([128, GPC * 6], F32)     # per-graph accum stats phase C
            st1 = hsb.tile([128, 6], F32)
            st2 = hsb.tile([128, 6], F32)
            sqscr = hsb.tile([128, 512], F32)
            relu_t = hsb.tile([128, 512], F32)
            y2 = hsb.tile([128, GPC * 256], F32R)
            z2a = hsb.tile([128, GPC * 256], F32R)
            z2b = hsb.tile([128, GPC * 256], F32R)
            ab1 = hsb.tile([128, 6], F32)
            ab2 = hsb.tile([128, 6], F32)

            def stats_into(ps_ap, cols, g, path):
                # Σ comes free with the ACT PSUM->SBUF copy (accum_out at the
                # call sites); Σ² is one ACT square-with-accumulate on the
                # f32 PSUM (exact — bf16-copy-based variance costs accuracy).
                c = g * 6 + 2 * path + 1
                nc.scalar.activation(
                    out=sqscr[:, :ps_ap.shape[-1]], in_=ps_ap,
                    func=AF.Square, accum_out=cols[:, c:c + 1])

            def stats_into_dve(ps_ap, sb_ap, cols, g, path):
                # DVE variant (offloads ACT in phase C): Σ² from PSUM x its
                # exact f32 SBUF copy with fused accumulate.
                c = g * 6 + 2 * path + 1
                nc.vector.scalar_tensor_tensor(
                    out=sqscr[:, :ps_ap.shape[-1]], in0=ps_ap, scalar=1.0,
                    in1=sb_ap, op0=ALU.bypass, op1=ALU.mult,
                    accum_out=cols[:, c:c + 1])

            def reduce_stats(cols, st):
                nc.vector.reduce_sum(
                    out=st[:, :6],
                    in_=cols[:].rearrange("p (g c) -> p c g", c=6),
                    axis=mybir.AxisListType.X)

            def bn_coeffs(st, col, g_col, b_col, nn, ab, acol):
                mean = hsb.tile([128, 1], F32, tag="bnm", name="bnm")
                nc.vector.tensor_scalar_mul(mean[:], st[:, col:col + 1], 1.0 / nn)
                var = hsb.tile([128, 1], F32, tag="bnv", name="bnv")
                nc.vector.tensor_scalar_mul(var[:], st[:, col + 1:col + 2], 1.0 / nn)
                msq = hsb.tile([128, 1], F32, tag="bnq", name="bnq")
                nc.vector.tensor_mul(out=msq[:], in0=mean[:], in1=mean[:])
                nc.vector.tensor_sub(out=var[:], in0=var[:], in1=msq[:])
                nc.vector.tensor_scalar_add(var[:], var[:], 1e-5)
                sd = hsb.tile([128, 1], F32, tag="bnsd", name="bnsd")
                nc.scalar.activation(out=sd[:], in_=var[:], func=AF.Sqrt)
                inv = hsb.tile([128, 1], F32, tag="bninv", name="bninv")
                nc.vector.reciprocal(out=inv[:], in_=sd[:])
                nc.vector.tensor_mul(out=ab[:, acol:acol + 1], in0=inv[:], in1=bnp[:, g_col:g_col + 1])
                nc.vector.tensor_mul(out=mean[:], in0=mean[:], in1=ab[:, acol:acol + 1])
                nc.vector.tensor_sub(out=ab[:, acol + 1:acol + 2], in0=bnp[:, b_col:b_col + 1], in1=mean[:])

            def bn_relu_pool3(src_ap, acol, out_ap, ab):
                # bn+relu then maxpool k=3 s=2: [*, Lp] -> [*, P1]
                nc.scalar.activation(out=relu_t[:, :Lp], in_=src_ap, func=AF.Relu,
                                     bias=ab[:, acol + 1:acol + 2], scale=ab[:, acol:acol + 1])
                a = relu_t[:, 0:2 * P1].rearrange("p (l t) -> p t l", t=2)
                bb = relu_t[:, 2:2 + 2 * P1].rearrange("p (l t) -> p t l", t=2)
                mx = hsb.tile([128, P1], F32, tag="mx", name="mx")
                nc.vector.tensor_max(out=mx[:], in0=a[:, 0, :], in1=a[:, 1, :])
                nc.vector.tensor_max(out=out_ap, in0=mx[:], in1=bb[:, 0, :])

            # ---- phase A/B: conv1+convc1, stats, bn+relu+pool ----
            with tc.tile_pool(name="pA_sb", bufs=1) as pa, \
                 tc.tile_pool(name="pA_ps", bufs=2, space="PSUM") as hps:
                c1w = pa.tile([128, 3 * 128], F32R)
                nc.gpsimd.dma_start(out=c1w[:].rearrange("a (k b) -> a k b", b=128), in_=c1w_in.rearrange("k a b -> a k b"))
                cc1w = pa.tile([128, 12 * 128], F32R)
                nc.gpsimd.dma_start(out=cc1w[:].rearrange("a (k b) -> a k b", b=128), in_=cc1w_in.rearrange("k a b -> a k b"))
                y1 = pa.tile([128, GPC * 512], BF16)
                z1a = pa.tile([128, GPC * 512], BF16)
                z1b = pa.tile([128, GPC * 512], BF16)

                for g in range(GPC):
                    gs = slice(g * 512, g * 512 + 512)
                    hg = hT[:, gs]
                    xg = xT[:, gs]
                    c1ps = hps.tile([128, 512], F32, tag="c1ps", name="c1ps")
                    for k in range(3):
                        nc.tensor.matmul(c1ps[:, :Lp], c1w[:, k * 128:(k + 1) * 128],
                                         hg[:, k:k + Lp], start=(k == 0), stop=(k == 2))
                    nc.scalar.activation(out=y1[:, g * 512:g * 512 + Lp], in_=c1ps[:, :Lp],
                                         func=AF.Copy, accum_out=stA[:, g * 6:g * 6 + 1])
                    stats_into(c1ps[:, :Lp], stA, g, 0)
                    for co in range(2):
                        ccps = hps.tile([128, 512], F32, tag="ccps", name="ccps")
                        for k in range(3):
                            nc.tensor.matmul(ccps[:, :Lp],
                                             cc1w[:, (k * 4 + co) * 128:(k * 4 + co) * 128 + 128],
                                             hg[:, k:k + Lp], start=(k == 0), stop=False)
                        for k in range(3):
                            nc.tensor.matmul(ccps[:, :Lp],
                                             cc1w[:, (k * 4 + 2 + co) * 128:(k * 4 + 2 + co) * 128 + 128],
                                             xg[:, k:k + Lp], start=False, stop=(k == 2))
                        dst = z1a if co == 0 else z1b
                        c = g * 6 + 2 * (1 + co)
                        nc.scalar.activation(out=dst[:, g * 512:g * 512 + Lp], in_=ccps[:, :Lp],
                                             func=AF.Copy, accum_out=stA[:, c:c + 1])
                        stats_into(ccps[:, :Lp], stA, g, 1 + co)

                reduce_stats(stA, st1)
                nc.sync.dma_start(out=ar1_in[:, :], in_=st1[:])
                nc.gpsimd.collective_compute("AllReduce", ALU.add, replica_groups=rg,
                                             ins=[ar1_in[:, :]], outs=[ar1_out[:, :]])
                nc.sync.dma_start(out=st1[:], in_=ar1_out[:, :])
                bn_coeffs(st1, 0, 0, 1, NN1, ab1, 0)
                bn_coeffs(st1, 2, 2, 3, NN1, ab1, 2)
                bn_coeffs(st1, 4, 4, 5, NN1, ab1, 4)

                for g in range(GPC):
                    gs = slice(g * 512, g * 512 + 512)
                    o = g * 256
                    bn_relu_pool3(y1[:, gs][:, :Lp], 0, y2[:, o:o + P1], ab1)
                    bn_relu_pool3(z1a[:, gs][:, :Lp], 2, z2a[:, o:o + P1], ab1)
                    bn_relu_pool3(z1b[:, gs][:, :Lp], 4, z2b[:, o:o + P1], ab1)

            # ---- phase C: conv2/convc2 + stats2 + bn/relu/pool + proj ----
            with tc.tile_pool(name="pC_sb", bufs=1) as pc, \
                 tc.tile_pool(name="pC_ps", bufs=2, space="PSUM") as hps:
                c2w = pc.tile([128, 128], F32R)
                nc.gpsimd.dma_start(out=c2w[:], in_=c2w_in[:, :])
                cc2w = pc.tile([128, 4 * 128], F32R)
                nc.gpsimd.dma_start(out=cc2w[:].rearrange("a (k b) -> a k b", b=128), in_=cc2w_in.rearrange("k a b -> a k b"))
                y3 = pc.tile([128, GPC * 256], F32)
                z3a = pc.tile([128, GPC * 256], F32)
                z3b = pc.tile([128, GPC * 256], F32)

                for g in range(GPC):
                    gs = slice(g * 256, g * 256 + 256)
                    c2ps = hps.tile([128, 256], F32, tag="c2ps", name="c2ps")
                    nc.tensor.matmul(c2ps[:], c2w[:], y2[:, gs], start=True, stop=True)
                    nc.scalar.activation(out=y3[:, gs][:, :P1], in_=c2ps[:, :P1],
                                         func=AF.Copy, accum_out=stC[:, g * 6:g * 6 + 1])
                    stats_into_dve(c2ps[:, :P1], y3[:, gs][:, :P1], stC, g, 0)
                    for co in range(2):
                        ccps2 = hps.tile([128, 256], F32, tag="ccps2", name="ccps2")
                        nc.tensor.matmul(ccps2[:], cc2w[:, co * 128:co * 128 + 128],
                                         z2a[:, gs], start=True, stop=False)
                        nc.tensor.matmul(ccps2[:], cc2w[:, (2 + co) * 128:(2 + co) * 128 + 128],
                                         z2b[:, gs], start=False, stop=True)
                        dst3 = z3a if co == 0 else z3b
                        c = g * 6 + 2 * (1 + co)
                        nc.scalar.activation(out=dst3[:, gs][:, :P1], in_=ccps2[:, :P1],
                                             func=AF.Copy, accum_out=stC[:, c:c + 1])
                        stats_into_dve(ccps2[:, :P1], dst3[:, gs][:, :P1], stC, g, 1 + co)

                reduce_stats(stC, st2)
                nc.sync.dma_start(out=ar2_in[:, :], in_=st2[:])
                nc.gpsimd.collective_compute("AllReduce", ALU.add, replica_groups=rg,
                                             ins=[ar2_in[:, :]], outs=[ar2_out[:, :]])
                nc.sync.dma_start(out=st2[:], in_=ar2_out[:, :])
                bn_coeffs(st2, 0, 0, 1, NN2, ab2, 0)
                bn_coeffs(st2, 2, 2, 3, NN2, ab2, 2)
                bn_coeffs(st2, 4, 4, 5, NN2, ab2, 4)

                mlpy = pc.tile([128, 2], F32R)
                nc.gpsimd.dma_start(out=mlpy[:], in_=mlpy_in[:, :])
                mlpz = pc.tile([128, 4], F32R)
                nc.gpsimd.dma_start(out=mlpz[:], in_=mlpz_in[:, :])
                mlpb = pc.tile([2, 2], F32)
                nc.sync.dma_start(out=mlpb[:], in_=mlpb_in[:, :])
                outsb = pc.tile([2, GPC], F32)
                y4 = pc.tile([128, GPC * 128], F32R)
                z4a = pc.tile([128, GPC * 128], F32R)
                z4b = pc.tile([128, GPC * 128], F32R)

                def bn_relu_pool2(src_t, gs, acol, out_ap, ab):
                    nc.scalar.activation(out=relu_t[:, :P1], in_=src_t[:, gs][:, :P1], func=AF.Relu,
                                         bias=ab[:, acol + 1:acol + 2], scale=ab[:, acol:acol + 1])
                    a = relu_t[:, 0:2 * L4].rearrange("p (l t) -> p t l", t=2)
                    nc.vector.tensor_max(out=out_ap, in0=a[:, 0, :], in1=a[:, 1, :])

                for g in range(GPC):
                    gs = slice(g * 256, g * 256 + 256)
                    bn_relu_pool2(y3, gs, 0, y4[:, g * 128:g * 128 + L4], ab2)
                    bn_relu_pool2(z3a, gs, 2, z4a[:, g * 128:g * 128 + L4], ab2)
                    bn_relu_pool2(z3b, gs, 4, z4b[:, g * 128:g * 128 + L4], ab2)

                # batched projection over 4-graph groups (512-col psum tiles)
                prod = pc.tile([128, GPC * 128], F32, tag="prod", name="prod")
                for q in range(GPC // 4):
                    qs = slice(q * 512, (q + 1) * 512)
                    yp = hps.tile([2, 512], F32, tag="yp", name="yp")
                    nc.tensor.matmul(yp[:], mlpy[:], y4[:, qs], start=True, stop=True)
                    zp = hps.tile([2, 512], F32, tag="zp", name="zp")
                    nc.tensor.matmul(zp[:], mlpz[:, 0:2], z4a[:, qs], start=True, stop=False)
                    nc.tensor.matmul(zp[:], mlpz[:, 2:4], z4b[:, qs], start=False, stop=True)
                    ypb = pc.tile([2, 512], F32, tag="ypb", name="ypb")
                    nc.vector.tensor_scalar_add(ypb[:], yp[:], mlpb[:, 0:1])
                    zpb = pc.tile([2, 512], F32, tag="zpb", name="zpb")
                    nc.vector.tensor_scalar_add(zpb[:], zp[:], mlpb[:, 1:2])
                    nc.vector.tensor_mul(out=prod[:2, qs], in0=ypb[:], in1=zpb[:])
                nc.vector.reduce_sum(
                    out=outsb[:, :GPC],
                    in_=prod[:2, :].rearrange("p (g l) -> p g l", l=128)[:, :, :L4],
                    axis=mybir.AxisListType.X)
                nc.vector.tensor_scalar_mul(outsb[:], outsb[:], 1.0 / L4)
                nc.sync.dma_start(out=out_p.rearrange("g p -> p g"), in_=outsb[:])

    nc.finalize()
    return nc


# --------------------------------------------------------------------------
# host weight packing
# --------------------------------------------------------------------------

def _make_inmaps(cfg, lay, inputs):
    N = cfg["N"]
    SH = N // NCORES
    f32 = np.float32
    x = np.asarray(inputs["x"], f32)
    wgg = np.ascontiguousarray(np.asarray(inputs["ggnn_w"], f32))
    wihT = np.ascontiguousarray(np.asarray(inputs["gru_wih"], f32).T)
    whhT = np.ascontiguousarray(np.asarray(inputs["gru_whh"], f32).T)
    bih = np.asarray(inputs["gru_bih"], f32)
    bhh = np.asarray(inputs["gru_bhh"], f32)
    gbias = np.zeros((128, 4), f32)
    gbias[:, 0] = bih[0:128] + bhh[0:128]
    gbias[:, 1] = bih[128:256] + bhh[128:256]
    gbias[:, 2] = bih[256:384]
    gbias[:, 3] = bhh[256:384]
    assert np.all(bhh[256:384] == 0), "nonzero bhh_n not supported"

    c1 = np.asarray(inputs["conv1_w"], f32)
    c1w = np.ascontiguousarray(np.transpose(c1, (2, 1, 0)))
    c2w = np.ascontiguousarray(np.asarray(inputs["conv2_w"], f32)[:, :, 0].T)
    cc1 = np.asarray(inputs["convc1_w"], f32)
    cc1w = np.zeros((12, 128, 128), f32)
    for k in range(3):
        for ci in range(2):
            for co in range(2):
                cc1w[k * 4 + ci * 2 + co] = cc1[co * 128:(co + 1) * 128,
                                                ci * 128:(ci + 1) * 128, k].T
    cc2 = np.asarray(inputs["convc2_w"], f32)[:, :, 0]
    cc2w = np.zeros((4, 128, 128), f32)
    for ci in range(2):
        for co in range(2):
            cc2w[ci * 2 + co] = cc2[co * 128:(co + 1) * 128, ci * 128:(ci + 1) * 128].T
    bnp = np.zeros((128, 6), f32)
    bnp[:, 0] = np.asarray(inputs["bn1_g"], f32)
    bnp[:, 1] = np.asarray(inputs["bn1_b"], f32)
    bn2g = np.asarray(inputs["bn2_g"], f32)
    bn2b = np.asarray(inputs["bn2_b"], f32)
    bnp[:, 2] = bn2g[:128]; bnp[:, 3] = bn2b[:128]
    bnp[:, 4] = bn2g[128:]; bnp[:, 5] = bn2b[128:]
    mlpyT = np.ascontiguousarray(np.asarray(inputs["mlpy_w"], f32).T)
    mzw = np.asarray(inputs["mlpz_w"], f32)
    mlpzT = np.zeros((128, 4), f32)
    mlpzT[:, 0:2] = mzw[:, :128].T
    mlpzT[:, 2:4] = mzw[:, 128:].T
    mlpb = np.zeros((2, 2), f32)
    mlpb[:, 0] = np.asarray(inputs["mlpy_b"], f32)
    mlpb[:, 1] = np.asarray(inputs["mlpz_b"], f32)

    common = dict(wgg=wgg, wihT=wihT, whhT=whhT, gbias=gbias, c1w=c1w, c2w=c2w,
                  cc1w=cc1w, cc2w=cc2w, bnp=bnp, mlpyT=mlpyT, mlpzT=mlpzT,
                  mlpb=mlpb)
    in_maps = []
    for c in range(NCORES):
        xT = np.ascontiguousarray(x[c * SH:(c + 1) * SH].T)
        in_maps.append(dict(xT=xT, gidx=lay["gidx"][c], ind=lay["ind"][c], **common))
    return in_maps


def run(cfg, inputs, trace=False):
    lay = _prep_edges(cfg, inputs["edge_index"], inputs["edge_weight"])
    nc = _build(cfg, lay)
    in_maps = _make_inmaps(cfg, lay, inputs)
    res = run_bass_kernel_spmd(nc, in_maps, list(range(NCORES)), trace=trace)
    out = np.concatenate([res.results[c]["out"] for c in range(NCORES)], axis=0)
    return out.astype(np.float32), res


def kernel(**inputs) -> np.ndarray:
    out, _ = run(_full_cfg(), inputs, trace=False)
    return out
